# revision 65
# baseline (speedup 1.0000x reference)
"""Trainium2 Bass kernel for nn_EuclideanToLorentzConv (8-core data-parallel).

v5 — fp16 matmul path, SWDGE bulk DMA, algebraic stsq, [28,896] pixel layout:
  * Conv as 9 window-matmuls in fp16 reading a padded SBUF-resident
    [128,114,114] image XP = [s | s^2] built once per image via DVE/ACT casts.
  * Bulk HBM traffic (x loads, output chunks) via gpsimd SWDGE dma_start
    (spreads across 16 SDMA engines); sync-ring HWDGE only carries small
    reshape DMAs (all HWDGE descriptors execute on SDMA engine 0).
  * Pixel-scalar fields live as [28, 896] (partition = half-band) so every
    pixel<->channel reshape DMA is 1-2 descriptors instead of 4.
  * Per-pixel stats via K=128 PE matmuls accumulated into one [3,448]
    PSUM tile per group; stsq has a closed form (no phase-2 tmp pass);
    tmp recomputed in fp32 psum in phase 3 via an identity matmul.
  * w2 per-pixel scale broadcast via K=1 PE matmul + copy to SBUF.
"""

import sys
import numpy as np
from contextlib import ExitStack

sys.path.insert(0, "/opt/trn_rl_repo")

import concourse.bass as bass  # noqa: E402
import concourse.tile as tile  # noqa: E402
from concourse import mybir, bacc  # noqa: E402
from concourse.bass_utils import run_bass_kernel_spmd  # noqa: E402

F32 = mybir.dt.float32
F16 = mybir.dt.float16
AX = mybir.AxisListType
OP = mybir.AluOpType
AF = mybir.ActivationFunctionType

# ---- problem constants (hardcoded; kernel.py must be self-contained) ----
NCORES = 8
B_GLOB, CIN, H, W = 16, 64, 112, 112
B_LOC = B_GLOB // NCORES            # 2 images per core
S = CIN - 1                         # 63 space channels in
M = 127                             # space channels out
COUT = M + 1
D = 9 * S + 1                       # 568
EPS = 1e-6

HP, WP = H + 2, W + 2               # padded 114x114
ROWS_PER_GROUP = 4
GROUP_PX = ROWS_PER_GROUP * W       # 448
BAND_ROWS = 16                      # output rows per band
GROUPS_PER_BAND = BAND_ROWS // ROWS_PER_GROUP   # 4
BANDS_PER_IMG = H // BAND_ROWS      # 7
NBANDS = B_LOC * BANDS_PER_IMG      # 14
NGROUPS = NBANDS * GROUPS_PER_BAND  # 56
NPX = NGROUPS * GROUP_PX            # 25088 pixels per core
NPX_GLOB = B_GLOB * H * W           # 200704

NPT = 2 * NBANDS                    # 28 pixel-tile partitions (half-band each)
PPX = NPX // NPT                    # 896 pixels per partition

CHUNK_ROWS = 7                      # x staging chunk (rows per chunk)
CHUNKS_PER_IMG = H // CHUNK_ROWS    # 8
OUT_BANDS = 2                       # output chunk = 2 bands -> 896B SWDGE descs
OUT_COLS = OUT_BANDS * BAND_ROWS * W  # 3584

_CACHE = {}


def _build_nc():
    nc = bacc.Bacc("TRN2", target_bir_lowering=False, debug=False,
                   num_devices=NCORES)

    x_in = nc.dram_tensor("x", [B_LOC, CIN, H, W], F32, kind="ExternalInput")
    w9_in = nc.dram_tensor("w9", [128, 9 * 128], F16, kind="ExternalInput")
    sw3_in = nc.dram_tensor("sw3", [128, 8], F16, kind="ExternalInput")
    redw_in = nc.dram_tensor("redw", [M, 3], F32, kind="ExternalInput")
    lr1_in = nc.dram_tensor("lr1i", [2, 128], F16, kind="ExternalInput")
    ident_in = nc.dram_tensor("identi", [128, 128], F16, kind="ExternalInput")
    gamma_in = nc.dram_tensor("gamma", [1], F32, kind="ExternalInput")
    out_d = nc.dram_tensor("out", [B_LOC, COUT, H, W], F32,
                           kind="ExternalOutput")

    if _CACHE.get("debug"):
        dbg_ycm = nc.dram_tensor("dbg_ycm", [128, NPX], F16, kind="ExternalOutput")
        dbg_ps1 = nc.dram_tensor("dbg_ps1", [5, NPT, PPX], F32, kind="ExternalOutput")
        dbg_ps2 = nc.dram_tensor("dbg_ps2", [6, NPT, PPX], F32, kind="ExternalOutput")
        dbg_mu = nc.dram_tensor("dbg_mu", [130], F32, kind="ExternalOutput")
    cc1_in = nc.dram_tensor("cc1_in", [130], F32)
    cc1_out = nc.dram_tensor("cc1_out", [130], F32, addr_space="Shared")
    cc2_in = nc.dram_tensor("cc2_in", [2], F32)
    cc2_out = nc.dram_tensor("cc2_out", [2], F32, addr_space="Shared")
    groups_all = [list(range(NCORES))]

    with tile.TileContext(nc) as tc, ExitStack() as ctx:
        sing = ctx.enter_context(tc.tile_pool(name="sing", bufs=1))
        ysqp = ctx.enter_context(tc.tile_pool(name="ysq", bufs=2))
        w2rp = ctx.enter_context(tc.tile_pool(name="w2rp", bufs=2))
        outp = ctx.enter_context(tc.tile_pool(name="outp", bufs=2))
        stgp = ctx.enter_context(tc.tile_pool(name="stg", bufs=2))
        stagep = ctx.enter_context(tc.tile_pool(name="stage", bufs=2))
        psy = ctx.enter_context(tc.tile_pool(name="psy", bufs=2, space="PSUM"))
        pss = ctx.enter_context(tc.tile_pool(name="pss", bufs=2, space="PSUM"))
        psb = pss
        pr2p = ctx.enter_context(tc.tile_pool(name="pr2", bufs=2, space="PSUM"))

        # ---- static SBUF ----
        W9B = sing.tile([128, 9, 128], F16)
        nc.sync.dma_start(out=W9B, in_=w9_in[:].rearrange("p (w m) -> p w m", w=9))
        # SW3 stat weights [128, 8]:
        #   col0 = [W0;0]   col1 = e127      col2 = 0    (applied to y')
        #   col3 = 0        col4 = 0         col5 = 256*[1;..;1;0]  ((y'/16)^2)
        #   col6 = [1;...;1;0] (O127Z)       col7 = 256*[1;..;1;0]
        SW3 = sing.tile([128, 8], F16)
        nc.sync.dma_start(out=SW3, in_=sw3_in[:])
        O127Z = SW3[:, 6:7]
        O256Z = SW3[:, 7:8]
        REDW = sing.tile([M, 3], F32)
        nc.sync.dma_start(out=REDW, in_=redw_in[:])
        LR1B = sing.tile([2, 128], F16)
        nc.sync.dma_start(out=LR1B, in_=lr1_in[:])
        IDENT = sing.tile([128, 128], F16)
        nc.sync.dma_start(out=IDENT, in_=ident_in[:])
        GAM = sing.tile([1, 1], F32)
        nc.sync.dma_start(out=GAM, in_=gamma_in[:].rearrange("(o c) -> o c", o=1))
        ONER = sing.tile([1, 128], F16)
        nc.vector.memset(ONER, 1.0)
        ONES28 = sing.tile([NPT, 1], F32)
        nc.vector.memset(ONES28, 1.0)
        BYT = sing.tile([NPT, 1], F32)
        nc.vector.memset(BYT, float(1.0 + _CACHE["c_w0sq"]))
        BM1 = sing.tile([NPT, 1], F32)
        nc.vector.memset(BM1, -1.0)
        BEPSV = sing.tile([1, 1], F32)
        nc.vector.memset(BEPSV, 1e-5)
        MUHSB = sing.tile([128, 1], F16)       # [mu_s;0] fp16, set after AR1
        nc.vector.memset(MUHSB, 0.0)

        YCMB = sing.tile([128, NPX], F16)      # rows 0..126 y', row 127 T^2-1
        MUP = sing.tile([128, NGROUPS], F32)   # per-group per-channel sums
        XP = sing.tile([128, HP, WP], F16)     # [s | s^2] padded image
        nc.vector.memset(XP, 0.0)

        # pixel-scalar fields, [28, 896] (partition = half-band)
        def ps(name, dt=F32):
            t = sing.tile([NPT, PPX], dt, tag=name, name=name)
            return t
        T2M1, TPS, W0DOT, YSQ1, YT = ps("t2m1"), ps("tps"), ps("w0dot"), ps("ysq1"), ps("yt")
        MUDOT, MA, ALPHA, FPS, HPS = ps("mudot"), ps("ma"), ps("alpha"), ps("fps"), ps("hps")
        PSA, PSB, PSC = ps("psa"), ps("psb"), ps("psc")
        TPSB, HPSB, W2B = ps("tpsb", F16), ps("hpsb", F16), ps("w2b", F16)
        YTSQ = ps("ytsq")

        # ================= PHASE 1: conv =================
        with nc.allow_low_precision("fp16 conv by design"):
            # stats matmuls trail one group behind the conv stream so the
            # in-order PE never stalls on the DVE evac / ACT square feeding them
            pend_st = [None]    # (g, cols, ysq) awaiting stats matmuls
            stga = {}           # band -> STGA staging tile

            def flush_stats():
                gp, colsp, ysq_t = pend_st[0]
                pend_st[0] = None
                bandp, kp = divmod(gp, GROUPS_PER_BAND)
                if kp == 0:
                    stga[bandp] = stgp.tile([3, GROUPS_PER_BAND, GROUP_PX],
                                            F32, tag="stg3", name="STGA")
                STGA = stga[bandp]
                psA = pss.tile([3, GROUP_PX], F32, tag="sb")
                nc.tensor.matmul(psA[:], lhsT=SW3[:, 0:3], rhs=YCMB[:, colsp],
                                 start=True, stop=False)
                nc.tensor.matmul(psA[:], lhsT=SW3[:, 3:6], rhs=ysq_t[:],
                                 start=False, stop=True)
                if kp % 2 == 0:
                    nc.vector.tensor_copy(out=STGA[:, kp, :], in_=psA[:])
                else:
                    nc.scalar.activation(out=STGA[:, kp, :], in_=psA[:],
                                         func=AF.Copy)
                if kp == GROUPS_PER_BAND - 1:
                    pslp = slice(2 * bandp, 2 * bandp + 2)
                    nc.sync.dma_start(out=W0DOT[pslp, :], in_=STGA[0:1, :, :])
                    nc.sync.dma_start(out=T2M1[pslp, :], in_=STGA[1:2, :, :])
                    nc.sync.dma_start(out=YSQ1[pslp, :], in_=STGA[2:3, :, :])
                    del stga[bandp]

            for b in range(B_LOC):
                # build XP = [s | s^2] fp16 with padding
                for q in range(CHUNKS_PER_IMG):
                    r0 = q * CHUNK_ROWS
                    stg = stagep.tile([128, CHUNK_ROWS, W], F32, tag="stg")
                    src = x_in[b, 1:CIN, r0:r0 + CHUNK_ROWS, :].rearrange(
                        "c h w -> c (h w)")
                    nc.gpsimd.dma_start(
                        out=stg[0:S].rearrange("c h w -> c (h w)"), in_=src)
                    nc.gpsimd.dma_start(
                        out=stg[64:64 + S].rearrange("c h w -> c (h w)"), in_=src)
                    nc.vector.tensor_scalar_add(
                        XP[0:S, 1 + r0:1 + r0 + CHUNK_ROWS, 1:1 + W],
                        stg[0:S], 0.0)
                    nc.scalar.activation(
                        out=XP[64:64 + S, 1 + r0:1 + r0 + CHUNK_ROWS, 1:1 + W],
                        in_=stg[64:64 + S], func=AF.Square)

                for rb in range(BANDS_PER_IMG):
                    band = b * BANDS_PER_IMG + rb
                    for k in range(GROUPS_PER_BAND):
                        g = band * GROUPS_PER_BAND + k
                        cols = bass.ts(g, GROUP_PX)
                        h0 = rb * BAND_ROWS + k * ROWS_PER_GROUP
                        psum = psy.tile([128, GROUP_PX], F32, tag="psy")
                        for wi in range(9):
                            i, j = divmod(wi, 3)
                            rhs = XP[:, h0 + i:h0 + i + ROWS_PER_GROUP, j:j + W]
                            nc.tensor.matmul(psum[:], lhsT=W9B[:, wi, :], rhs=rhs,
                                             start=(wi == 0), stop=(wi == 8))
                        # evacuate to fp16 + per-channel partial sums (for mu)
                        nc.vector.tensor_scalar(out=YCMB[:, cols], in0=psum[:],
                                                scalar1=0.0, scalar2=None, op0=OP.add,
                                                op1=OP.add, accum_out=MUP[:, g:g + 1])
                        ysq = ysqp.tile([128, GROUP_PX], F16, tag="ysq")
                        nc.scalar.activation(out=ysq, in_=psum[:], func=AF.Square,
                                             scale=0.0625)
                        if pend_st[0] is not None:
                            flush_stats()
                        pend_st[0] = (g, cols, ysq)
            if pend_st[0] is not None:
                flush_stats()

        # ---- pixel-scalar chain, phase 1 ----
        # T = sqrt(1 + T2m1)
        nc.scalar.activation(out=TPS, in_=T2M1, func=AF.Sqrt, bias=1.0)
        # ysqf = ysq1 + 2*T*w0dot + T2m1*c_w0sq ; y_t = sqrt(1 + c_w0sq + ysqf')
        nc.vector.tensor_mul(PSA, TPS, W0DOT)
        nc.vector.scalar_tensor_tensor(out=PSB, in0=PSA, scalar=2.0, in1=YSQ1,
                                       op0=OP.mult, op1=OP.add)
        nc.vector.scalar_tensor_tensor(out=PSC, in0=T2M1, scalar=_CACHE["c_w0sq"],
                                       in1=PSB, op0=OP.mult, op1=OP.add)
        nc.scalar.activation(out=YT, in_=PSC, func=AF.Sqrt, bias=BYT[:])
        nc.vector.tensor_mul(YTSQ, YT, YT)
        # reduced sums for the collective
        MUS = sing.tile([128, 1], F32)
        nc.vector.tensor_reduce(MUS, MUP, axis=AX.X, op=OP.add)
        SR = sing.tile([NPT, 2], F32)
        nc.vector.tensor_reduce(SR[:, 0:1], TPS, axis=AX.X, op=OP.add)
        nc.vector.tensor_reduce(SR[:, 1:2], YT, axis=AX.X, op=OP.add)
        pt = psb.tile([1, GROUP_PX], F32, tag="sb")
        nc.tensor.matmul(pt[0:1, 0:2], lhsT=ONES28, rhs=SR[:], start=True, stop=True)
        SC0 = sing.tile([1, 2], F32)
        nc.vector.tensor_copy(out=SC0, in_=pt[0:1, 0:2])
        nc.sync.dma_start(out=cc1_in[0:128], in_=MUS)
        nc.sync.dma_start(out=cc1_in[128:130], in_=SC0)
        nc.gpsimd.collective_compute("AllReduce", OP.add, replica_groups=groups_all,
                                     ins=[cc1_in[:]], outs=[cc1_out[:]])
        # keep the PE HAM-warm through the collective wait (results unused)
        with nc.allow_low_precision("warmup"):
            for _ in range(50):
                wps = psy.tile([128, GROUP_PX], F32, tag="psy")
                nc.tensor.matmul(wps[:], lhsT=IDENT, rhs=YCMB[:, 0:GROUP_PX],
                                 start=True, stop=True)
        MUSG = sing.tile([128, 1], F32)
        nc.sync.dma_start(out=MUSG, in_=cc1_out[0:128].rearrange("(p o) -> p o", o=1))
        SC0G = sing.tile([1, 2], F32)
        nc.sync.dma_start(out=SC0G, in_=cc1_out[128:130].rearrange("(o c) -> o c", o=1))

        # ---- mu normalization (tiny ops) ----
        invN = 1.0 / float(NPX_GLOB)
        SC127 = sing.tile([M, 2], F32)
        nc.gpsimd.partition_broadcast(SC127, SC0G)
        MUUS = sing.tile([M, 1], F32)      # unnormalized mean of y_s
        nc.vector.scalar_tensor_tensor(out=MUUS, in0=REDW[:, 0:1],
                                       scalar=SC127[:, 0:1], in1=MUSG[0:M, :],
                                       op0=OP.mult, op1=OP.add)
        nc.vector.tensor_scalar_mul(MUUS, MUUS, invN)
        MU0U = sing.tile([1, 1], F32)
        nc.vector.tensor_scalar_mul(MU0U, SC0G[0:1, 1:2], invN)
        MSQ = sing.tile([M, 1], F32)
        nc.vector.tensor_mul(MSQ, MUUS, MUUS)
        pt2 = psb.tile([1, GROUP_PX], F32, tag="sb")
        nc.tensor.matmul(pt2[0:1, 0:1], lhsT=REDW[:, 2:3], rhs=MSQ[:],
                         start=True, stop=True)
        SMSQ = sing.tile([1, 1], F32)
        nc.vector.tensor_copy(out=SMSQ, in_=pt2[0:1, 0:1])
        T1 = sing.tile([1, 1], F32)
        nc.vector.tensor_mul(T1, MU0U, MU0U)
        nc.vector.tensor_sub(T1, T1, SMSQ)
        nc.scalar.activation(out=T1, in_=T1, func=AF.Sqrt)     # nrm
        RNRM = sing.tile([1, 1], F32)
        nc.vector.reciprocal(RNRM, T1)
        RN127 = sing.tile([M, 1], F32)
        nc.gpsimd.partition_broadcast(RN127, RNRM)
        MUHS = sing.tile([M, 1], F32)
        nc.vector.tensor_scalar_mul(MUHS, MUUS, RN127[:, 0:1])
        MU0H = sing.tile([1, 1], F32)
        nc.vector.tensor_mul(MU0H, MU0U, RNRM)
        # c_muW0 = sum(mu_s * W0)
        PRD = sing.tile([M, 1], F32)
        nc.vector.tensor_mul(PRD, MUHS, REDW[:, 0:1])
        pt3 = psb.tile([1, GROUP_PX], F32, tag="sb")
        nc.tensor.matmul(pt3[0:1, 0:1], lhsT=REDW[:, 2:3], rhs=PRD[:],
                         start=True, stop=True)
        # inv1p = 1/(1+mu0)
        INV1P = sing.tile([1, 1], F32)
        nc.vector.tensor_scalar_add(INV1P, MU0H, 1.0)
        nc.vector.reciprocal(INV1P, INV1P)
        # scalar bundle -> 28 partitions: {mu0, inv1p, c_muW0, mu0^2-1}
        SCROW = sing.tile([1, 4], F32)
        nc.vector.tensor_copy(out=SCROW[:, 0:1], in_=MU0H)
        nc.vector.tensor_copy(out=SCROW[:, 1:2], in_=INV1P)
        nc.vector.tensor_copy(out=SCROW[:, 2:3], in_=pt3[0:1, 0:1])
        nc.vector.tensor_mul(SCROW[:, 3:4], MU0H, MU0H)
        nc.vector.tensor_scalar_add(SCROW[:, 3:4], SCROW[:, 3:4], -1.0)
        SC28 = sing.tile([NPT, 4], F32)
        nc.gpsimd.partition_broadcast(SC28, SCROW)
        # fp16 casts of mu for phase-2/3 matmuls
        with nc.allow_low_precision("fp16 mu by design"):
            nc.vector.tensor_scalar_mul(MUHSB[0:M], MUHS, 1.0)
            NMUB = sing.tile([M, 1], F16)
            nc.vector.tensor_scalar_mul(NMUB, MUHS, -1.0)
        # LR1B row0 = -mu_s  (tiny transposing DMA [127,1] -> [1,127])
        nc.sync.dma_start(out=LR1B[0:1, 0:M], in_=NMUB[:])

        if _CACHE.get("debug"):
            nc.sync.dma_start(out=dbg_ycm[:], in_=YCMB[:])
            for i_, t_ in enumerate([T2M1, TPS, W0DOT, YSQ1, YT]):
                nc.sync.dma_start(out=dbg_ps1[i_], in_=t_[:])
            nc.sync.dma_start(out=dbg_mu[0:128], in_=MUSG[:])
            nc.sync.dma_start(out=dbg_mu[128:130], in_=SC0G[:])

        # ================= PHASE 2 =================
        with nc.allow_low_precision("fp16 phase2 by design"):
            for band in range(NBANDS):
                psl = slice(2 * band, 2 * band + 2)
                STGC = stgp.tile([3, GROUPS_PER_BAND, GROUP_PX], F32, tag="stg3")
                for k in range(GROUPS_PER_BAND):
                    g = band * GROUPS_PER_BAND + k
                    cols = bass.ts(g, GROUP_PX)
                    psm = psb.tile([1, GROUP_PX], F32, tag="sb")
                    nc.tensor.matmul(psm[:], lhsT=MUHSB, rhs=YCMB[:, cols],
                                     start=True, stop=True)
                    if k % 2 == 0:
                        nc.vector.tensor_copy(out=STGC[0:1, k, :], in_=psm[:])
                    else:
                        nc.scalar.activation(out=STGC[0:1, k, :], in_=psm[:],
                                             func=AF.Copy)
                nc.sync.dma_start(out=MUDOT[psl, :], in_=STGC[0:1, :, :])

            # alpha = clip(mu0*yt - (mudot + T*c_muW0), 1+eps)
            nc.vector.scalar_tensor_tensor(out=MA, in0=TPS, scalar=SC28[:, 2:3],
                                           in1=MUDOT, op0=OP.mult, op1=OP.add)
            nc.vector.scalar_tensor_tensor(out=ALPHA, in0=YT, scalar=SC28[:, 0:1],
                                           in1=MA, op0=OP.mult, op1=OP.subtract)
            nc.vector.tensor_scalar_max(ALPHA, ALPHA, 1.0 + EPS)
            # f = ln(alpha + sqrt(alpha^2-1)) / sqrt(alpha^2-1)   (on DVE+ACT)
            # H = alpha + (yt - alpha*mu0) * inv1p                (on GPSIMD)
            nc.vector.tensor_scalar(out=PSC, in0=ALPHA, scalar1=SC28[:, 0:1],
                                    scalar2=None, op0=OP.mult)
            nc.vector.tensor_mul(PSA, ALPHA, ALPHA)
            nc.scalar.activation(out=PSB, in_=PSA, func=AF.Sqrt, bias=BM1[:])
            nc.gpsimd.tensor_sub(PSC, YT, PSC)
            nc.vector.scalar_tensor_tensor(out=HPS, in0=PSC, scalar=SC28[:, 1:2],
                                           in1=ALPHA, op0=OP.mult, op1=OP.add)
            nc.vector.tensor_add(PSA, ALPHA, PSB)
            nc.scalar.activation(out=PSA, in_=PSA, func=AF.Ln)
            nc.vector.reciprocal_approx_fast(out=FPS, in_=PSB)
            nc.vector.tensor_mul(FPS, FPS, PSA)
            nc.gpsimd.tensor_copy(out=HPSB, in_=HPS)
            nc.scalar.activation(out=TPSB, in_=TPS, func=AF.Copy)
            STSQ = ALPHA    # alpha dead after HPS; reuse its tile
            # stsq = YT^2 - 1 + H*(H*(mu0^2-1) - 2*MA)
            nc.vector.tensor_scalar(out=PSC, in0=HPS, scalar1=SC28[:, 3:4],
                                    scalar2=None, op0=OP.mult)
            nc.vector.scalar_tensor_tensor(out=PSC, in0=MA, scalar=-2.0,
                                           in1=PSC, op0=OP.mult, op1=OP.add)
            nc.gpsimd.tensor_mul(PSC, PSC, HPS)
            nc.vector.tensor_add(STSQ, PSC, YTSQ)
            nc.vector.tensor_scalar_add(STSQ, STSQ, -1.0)

        if _CACHE.get("debug"):
            for i_, t_ in enumerate([MUDOT, FPS, HPS, STSQ]):
                nc.sync.dma_start(out=dbg_ps2[i_], in_=t_[:])

        # var = mean(f^2 * stsq)  -> allreduce
        nc.vector.tensor_mul(PSA, FPS, FPS)
        nc.vector.tensor_mul(PSB, PSA, STSQ)
        VR = sing.tile([NPT, 1], F32)
        nc.vector.tensor_reduce(VR, PSB, axis=AX.X, op=OP.add)
        pt4 = psb.tile([1, GROUP_PX], F32, tag="sb")
        nc.tensor.matmul(pt4[0:1, 0:1], lhsT=ONES28, rhs=VR[:], start=True, stop=True)
        VSC = sing.tile([1, 2], F32)
        nc.vector.tensor_copy(out=VSC[:, 0:1], in_=pt4[0:1, 0:1])
        nc.vector.tensor_copy(out=VSC[:, 1:2], in_=pt4[0:1, 0:1])
        nc.sync.dma_start(out=cc2_in[:], in_=VSC)
        nc.gpsimd.collective_compute("AllReduce", OP.add, replica_groups=groups_all,
                                     ins=[cc2_in[:]], outs=[cc2_out[:]])
        with nc.allow_low_precision("warmup"):
            for _ in range(30):
                wps = psy.tile([128, GROUP_PX], F32, tag="psy")
                nc.tensor.matmul(wps[:], lhsT=IDENT, rhs=YCMB[:, 0:GROUP_PX],
                                 start=True, stop=True)
        # w2 = gf*sinh(vn)/vn with vn = gf*u, u = sqrt(stsq)  =>  w2 = sinh(vn)/u.
        # u, 0.5/u, f*u are g-independent: compute them during the AR2 wait.
        U = W0DOT
        INVU = YSQ1
        FUU = PSA
        nc.vector.tensor_scalar_max(PSC, STSQ, 1e-8)
        nc.scalar.activation(out=U, in_=PSC, func=AF.Sqrt)
        nc.vector.reciprocal_approx_fast(out=INVU, in_=U)
        nc.vector.tensor_scalar_mul(INVU, INVU, 0.5)
        nc.vector.tensor_mul(FUU, FPS, U)
        VG = sing.tile([1, 2], F32)
        nc.sync.dma_start(out=VG, in_=cc2_out[:].rearrange("(o c) -> o c", o=1))
        GSC = sing.tile([1, 1], F32)
        nc.vector.tensor_scalar_mul(GSC, VG[0:1, 0:1], invN)
        nc.scalar.activation(out=GSC, in_=GSC, func=AF.Sqrt, bias=BEPSV[:])
        nc.vector.reciprocal(GSC, GSC)
        nc.vector.tensor_mul(GSC, GSC, GAM)
        G28 = sing.tile([NPT, 1], F32)
        nc.gpsimd.partition_broadcast(G28, GSC)

        # ================= PHASE 3 =================
        # vn = g*(f*u); w2 = sinh(vn)/u = (exp(vn)-exp(-vn)) * (0.5/u)
        VN = TPS
        nc.vector.tensor_scalar(out=VN, in0=FUU, scalar1=G28[:, 0:1],
                                scalar2=None, op0=OP.mult)
        EXT = STSQ
        nc.scalar.activation(out=EXT, in_=VN, func=AF.Exp)
        nc.scalar.activation(out=PSC, in_=VN, func=AF.Exp, scale=-1.0)
        nc.vector.tensor_sub(EXT, EXT, PSC)                         # 2*sinh
        W2 = MUDOT
        nc.vector.tensor_mul(W2, EXT, INVU)
        with nc.allow_low_precision("fp16 w2 by design"):
            nc.scalar.activation(out=W2B, in_=W2, func=AF.Copy)
        if _CACHE.get("debug"):
            nc.sync.dma_start(out=dbg_ps2[5], in_=W2[:])

        out_flat = [out_d[b_, 1:COUT].rearrange("c h w -> c (h w)")
                    for b_ in range(B_LOC)]

        RSQ2 = MA    # mu-dot accumulator tile is dead after stsq; reuse

        def stage_band(band):
            psl = slice(2 * band, 2 * band + 2)
            HT = stgp.tile([2, GROUPS_PER_BAND, GROUP_PX], F16, tag="htb")
            nc.sync.dma_start(out=HT[0:1, :, :], in_=HPSB[psl, :])
            nc.sync.dma_start(out=HT[1:2, :, :], in_=TPSB[psl, :])
            W2S = stgp.tile([1, GROUPS_PER_BAND, GROUP_PX], F16, tag="w2s")
            nc.sync.dma_start(out=W2S[0:1, :, :], in_=W2B[psl, :])
            W2R = w2rp.tile([128, GROUPS_PER_BAND, GROUP_PX], F16, tag="w2r")
            nc.gpsimd.partition_broadcast(W2R, W2S[0:1, :, :])
            return HT, W2R

        with nc.allow_low_precision("fp16 phase3 by design"):
            nxt = stage_band(0)
            OUTCH = None
            for band in range(NBANDS):
                b, rb = divmod(band, BANDS_PER_IMG)
                psl = slice(2 * band, 2 * band + 2)
                HT, W2R = nxt
                if band + 1 < NBANDS:
                    nxt = stage_band(band + 1)
                ch = rb % OUT_BANDS          # position within output chunk
                if ch == 0:
                    OUTCH = outp.tile([128, OUT_COLS], F32, tag="outch")
                STGE = stgp.tile([3, GROUPS_PER_BAND, GROUP_PX], F32, tag="stg3")
                for blk in range(2):
                    pr2 = pr2p.tile([128, 2, 512], F32, tag="pr2")
                    for j in range(2):
                        k = blk * 2 + j
                        g = band * GROUPS_PER_BAND + k
                        cols = bass.ts(g, GROUP_PX)
                        nc.tensor.matmul(pr2[:, j, 0:GROUP_PX], lhsT=LR1B,
                                         rhs=HT[:, k, :], start=True, stop=False)
                        nc.tensor.matmul(pr2[:, j, 0:GROUP_PX], lhsT=IDENT,
                                         rhs=YCMB[:, cols], start=False, stop=True)
                    bsl = slice((ch * 4 + blk * 2) * GROUP_PX,
                                (ch * 4 + blk * 2 + 2) * GROUP_PX)
                    nc.vector.scalar_tensor_tensor(
                        out=OUTCH[:, bsl].rearrange("p (b c) -> p b c", b=2),
                        in0=pr2[:, :, 0:GROUP_PX], scalar=0.0,
                        in1=W2R[:, blk * 2:blk * 2 + 2, :],
                        op0=OP.max, op1=OP.mult)
                    sqo = ysqp.tile([128, 2 * GROUP_PX], F16, tag="sqo")
                    nc.scalar.activation(out=sqo, in_=OUTCH[:, bsl],
                                         func=AF.Square, scale=0.0625)
                    for j in range(2):
                        k = blk * 2 + j
                        psr = psb.tile([1, GROUP_PX], F32, tag="sb")
                        nc.tensor.matmul(psr[:], lhsT=O256Z,
                                         rhs=sqo[:, bass.ts(j, GROUP_PX)],
                                         start=True, stop=True)
                        if k % 2 == 0:
                            nc.scalar.activation(out=STGE[0:1, k, :], in_=psr[:],
                                                 func=AF.Copy)
                        else:
                            nc.vector.tensor_copy(out=STGE[0:1, k, :], in_=psr[:])
                nc.sync.dma_start(out=RSQ2[psl, :], in_=STGE[0:1, :, :])
                if ch == OUT_BANDS - 1 or rb == BANDS_PER_IMG - 1:
                    ncols = (ch + 1) * BAND_ROWS * W
                    col0 = (rb - ch) * BAND_ROWS * W
                    dst = out_flat[b][:, col0:col0 + ncols]
                    for i in range(8):
                        p0 = i * 16
                        p1 = min(M, p0 + 16)
                        eng = nc.gpsimd if i < 4 else (nc.sync if i < 6 else nc.scalar)
                        eng.dma_start(out=dst[p0:p1, :],
                                      in_=OUTCH[p0:p1, 0:ncols])

        # rt = sqrt(1 + sum rs^2) -> channel 0 plane
        RT = T2M1
        nc.scalar.activation(out=RT, in_=RSQ2, func=AF.Sqrt, bias=1.0)
        nc.gpsimd.dma_start(out=out_d[:, 0, :, :], in_=RT)

    nc.compile()
    return nc


def _prep_consts(W):
    W = np.asarray(W, np.float32)
    f16 = np.float16
    w9 = np.zeros((128, 9, 128), np.float32)
    for wi in range(9):
        w9[0:S, wi, 0:M] = W[:, 1 + wi * S:1 + (wi + 1) * S].T
        w9[64:64 + S, wi, 127] = 1.0
    sw3 = np.zeros((128, 8), np.float32)
    sw3[0:M, 0] = W[:, 0]
    sw3[127, 1] = 1.0
    sw3[0:M, 5] = 256.0
    sw3[0:M, 6] = 1.0
    sw3[0:M, 7] = 256.0
    redw = np.zeros((M, 3), np.float32)
    redw[:, 0] = W[:, 0]
    redw[:, 2] = 1.0
    lr1 = np.zeros((2, 128), np.float32)
    lr1[1, 0:M] = W[:, 0]
    ident = np.eye(128, dtype=np.float32)
    c_w0sq = float(np.float32((W[:, 0].astype(np.float64) ** 2).sum()))
    return (w9.reshape(128, 9 * 128).astype(f16), sw3.astype(f16), redw,
            lr1.astype(f16), ident.astype(f16), c_w0sq)


def _in_maps(x, W, gamma):
    x = np.ascontiguousarray(np.asarray(x, np.float32))
    gamma = np.asarray(gamma, np.float32)
    w9, sw3, redw, lr1, ident, c_w0sq = _prep_consts(W)
    if "nc" not in _CACHE:
        _CACHE["c_w0sq"] = c_w0sq
        _CACHE["nc"] = _build_nc()
    maps = []
    for c in range(NCORES):
        maps.append({
            "x": x[c * B_LOC:(c + 1) * B_LOC],
            "w9": w9, "sw3": sw3, "redw": redw, "lr1i": lr1, "identi": ident,
            "gamma": gamma,
        })
    return _CACHE["nc"], maps


def kernel(x, W, gamma, beta):
    beta = np.asarray(beta, np.float32)
    gamma = np.asarray(gamma, np.float32)
    assert abs(float(beta[0]) - 1.0) < 1e-6 and np.all(np.abs(beta[1:]) < 1e-6), \
        "kernel specialized for beta == Lorentz origin"
    assert float(gamma[0]) > 0.0
    nc, in_maps = _in_maps(x, W, gamma)
    res = run_bass_kernel_spmd(nc, in_maps, list(range(NCORES)))
    out = np.concatenate([res.results[c]["out"] for c in range(NCORES)], axis=0)
    return out


def run_traced(inputs, tmpdir=None):
    """Run with NTFF tracing; returns (exec_time_ns, BassKernelResults)."""
    nc, in_maps = _in_maps(inputs["x"], inputs["W"], inputs["gamma"])
    res = run_bass_kernel_spmd(nc, in_maps, list(range(NCORES)),
                               trace=True, tmpdir=tmpdir)
    return res.exec_time_ns, res


def simulate(inputs, debug=True):
    """Run the kernel through MultiCoreSim; returns list of per-core output dicts."""
    from concourse.bass_interp import MultiCoreSim
    _CACHE.clear()
    _CACHE["debug"] = debug
    x = np.asarray(inputs["x"], np.float32)
    w9, sw3, redw, lr1, ident, c_w0sq = _prep_consts(inputs["W"])
    _CACHE["c_w0sq"] = c_w0sq
    nc = _build_nc()
    sim = MultiCoreSim(nc, num_cores=NCORES)
    for c in range(NCORES):
        cs = sim.cores[c]
        cs.tensor("x")[:] = x[c * B_LOC:(c + 1) * B_LOC]
        cs.tensor("w9")[:] = w9
        cs.tensor("sw3")[:] = sw3
        cs.tensor("redw")[:] = redw
        cs.tensor("lr1i")[:] = lr1
        cs.tensor("identi")[:] = ident
        cs.tensor("gamma")[:] = np.asarray(inputs["gamma"], np.float32)
    sim.simulate(check_with_hw=False)
    names = ["out"]
    if debug:
        names += ["dbg_ycm", "dbg_ps1", "dbg_ps2", "dbg_mu"]
    return [{n: np.array(sim.cores[c].tensor(n)) for n in names}
            for c in range(NCORES)]


if __name__ == "__main__":
    rng = np.random.default_rng(0)
    x = rng.standard_normal((B_GLOB, CIN, H, W), dtype=np.float32)
    W_ = (rng.standard_normal((M, D), dtype=np.float32) / np.sqrt(D)).astype(np.float32)
    gamma = np.ones((1,), np.float32)
    beta = np.zeros((COUT,), np.float32); beta[0] = 1.0
    out = kernel(x=x, W=W_, gamma=gamma, beta=beta)
    print("out", out.shape, out.dtype, np.abs(out).max())


# revision 73
# speedup vs baseline: 1.0137x; 1.0137x over previous
"""Trainium2 Bass kernel for nn_EuclideanToLorentzConv (8-core data-parallel).

v5 — fp16 matmul path, SWDGE bulk DMA, algebraic stsq, [28,896] pixel layout:
  * Conv as 9 window-matmuls in fp16 reading a padded SBUF-resident
    [128,114,114] image XP = [s | s^2] built once per image via DVE/ACT casts.
  * Bulk HBM traffic (x loads, output chunks) via gpsimd SWDGE dma_start
    (spreads across 16 SDMA engines); sync-ring HWDGE only carries small
    reshape DMAs (all HWDGE descriptors execute on SDMA engine 0).
  * Pixel-scalar fields live as [28, 896] (partition = half-band) so every
    pixel<->channel reshape DMA is 1-2 descriptors instead of 4.
  * Per-pixel stats via K=128 PE matmuls accumulated into one [3,448]
    PSUM tile per group; stsq has a closed form (no phase-2 tmp pass);
    tmp recomputed in fp32 psum in phase 3 via an identity matmul.
  * w2 per-pixel scale broadcast via K=1 PE matmul + copy to SBUF.
"""

import sys
import numpy as np
from contextlib import ExitStack

sys.path.insert(0, "/opt/trn_rl_repo")

import concourse.bass as bass  # noqa: E402
import concourse.tile as tile  # noqa: E402
from concourse import mybir, bacc  # noqa: E402
from concourse.bass_utils import run_bass_kernel_spmd  # noqa: E402

F32 = mybir.dt.float32
F16 = mybir.dt.float16
AX = mybir.AxisListType
OP = mybir.AluOpType
AF = mybir.ActivationFunctionType

# ---- problem constants (hardcoded; kernel.py must be self-contained) ----
NCORES = 8
B_GLOB, CIN, H, W = 16, 64, 112, 112
B_LOC = B_GLOB // NCORES            # 2 images per core
S = CIN - 1                         # 63 space channels in
M = 127                             # space channels out
COUT = M + 1
D = 9 * S + 1                       # 568
EPS = 1e-6

HP, WP = H + 2, W + 2               # padded 114x114
ROWS_PER_GROUP = 4
GROUP_PX = ROWS_PER_GROUP * W       # 448
BAND_ROWS = 16                      # output rows per band
GROUPS_PER_BAND = BAND_ROWS // ROWS_PER_GROUP   # 4
BANDS_PER_IMG = H // BAND_ROWS      # 7
NBANDS = B_LOC * BANDS_PER_IMG      # 14
NGROUPS = NBANDS * GROUPS_PER_BAND  # 56
NPX = NGROUPS * GROUP_PX            # 25088 pixels per core
NPX_GLOB = B_GLOB * H * W           # 200704

NPT = 2 * NBANDS                    # 28 pixel-tile partitions (half-band each)
PPX = NPX // NPT                    # 896 pixels per partition

CHUNK_ROWS = 7                      # x staging chunk (rows per chunk)
CHUNKS_PER_IMG = H // CHUNK_ROWS    # 8
OUT_BANDS = 2                       # output chunk = 2 bands -> 896B SWDGE descs
OUT_COLS = OUT_BANDS * BAND_ROWS * W  # 3584

_CACHE = {}


def _build_nc():
    nc = bacc.Bacc("TRN2", target_bir_lowering=False, debug=False,
                   num_devices=NCORES)

    x_in = nc.dram_tensor("x", [B_LOC, CIN, H, W], F32, kind="ExternalInput")
    w9_in = nc.dram_tensor("w9", [128, 9 * 128], F16, kind="ExternalInput")
    sw3_in = nc.dram_tensor("sw3", [128, 8], F16, kind="ExternalInput")
    redw_in = nc.dram_tensor("redw", [M, 3], F32, kind="ExternalInput")
    w0row_in = nc.dram_tensor("w0row", [1, 128], F32, kind="ExternalInput")
    lr1_in = nc.dram_tensor("lr1i", [2, 128], F16, kind="ExternalInput")
    ident_in = nc.dram_tensor("identi", [128, 128], F16, kind="ExternalInput")
    gamma_in = nc.dram_tensor("gamma", [1], F32, kind="ExternalInput")
    out_d = nc.dram_tensor("out", [B_LOC, COUT, H, W], F32,
                           kind="ExternalOutput")

    if _CACHE.get("debug"):
        dbg_ycm = nc.dram_tensor("dbg_ycm", [128, NPX], F16, kind="ExternalOutput")
        dbg_ps1 = nc.dram_tensor("dbg_ps1", [5, NPT, PPX], F32, kind="ExternalOutput")
        dbg_ps2 = nc.dram_tensor("dbg_ps2", [6, NPT, PPX], F32, kind="ExternalOutput")
        dbg_mu = nc.dram_tensor("dbg_mu", [130], F32, kind="ExternalOutput")
    cc1_in = nc.dram_tensor("cc1_in", [130], F32)
    cc1_out = nc.dram_tensor("cc1_out", [130], F32, addr_space="Shared")
    cc2_in = nc.dram_tensor("cc2_in", [2], F32)
    cc2_out = nc.dram_tensor("cc2_out", [2], F32, addr_space="Shared")
    groups_all = [list(range(NCORES))]

    with tile.TileContext(nc) as tc, ExitStack() as ctx:
        sing = ctx.enter_context(tc.tile_pool(name="sing", bufs=1))
        ysqp = ctx.enter_context(tc.tile_pool(name="ysq", bufs=2))
        w2rp = ctx.enter_context(tc.tile_pool(name="w2rp", bufs=2))
        outp = ctx.enter_context(tc.tile_pool(name="outp", bufs=2))
        stgp = ctx.enter_context(tc.tile_pool(name="stg", bufs=2))
        stagep = ctx.enter_context(tc.tile_pool(name="stage", bufs=2))
        psy = ctx.enter_context(tc.tile_pool(name="psy", bufs=2, space="PSUM"))
        pss = ctx.enter_context(tc.tile_pool(name="pss", bufs=2, space="PSUM"))
        psb = pss
        pr2p = ctx.enter_context(tc.tile_pool(name="pr2", bufs=2, space="PSUM"))

        # ---- static SBUF ----
        W9B = sing.tile([128, 9, 128], F16)
        nc.sync.dma_start(out=W9B, in_=w9_in[:].rearrange("p (w m) -> p w m", w=9))
        # SW3 stat weights [128, 8]:
        #   col0 = [W0;0]   col1 = e127      col2 = 0    (applied to y')
        #   col3 = 0        col4 = 0         col5 = 256*[1;..;1;0]  ((y'/16)^2)
        #   col6 = [1;...;1;0] (O127Z)       col7 = 256*[1;..;1;0]
        SW3 = sing.tile([128, 8], F16)
        nc.sync.dma_start(out=SW3, in_=sw3_in[:])
        O127Z = SW3[:, 6:7]
        O256Z = SW3[:, 7:8]
        REDW = sing.tile([M, 3], F32)
        nc.sync.dma_start(out=REDW, in_=redw_in[:])
        W0ROW = sing.tile([1, 128], F32)
        nc.sync.dma_start(out=W0ROW, in_=w0row_in[:])
        LR1B = sing.tile([2, 128], F16)
        nc.sync.dma_start(out=LR1B, in_=lr1_in[:])
        IDENT = sing.tile([128, 128], F16)
        nc.sync.dma_start(out=IDENT, in_=ident_in[:])
        GAM = sing.tile([1, 1], F32)
        nc.sync.dma_start(out=GAM, in_=gamma_in[:].rearrange("(o c) -> o c", o=1))
        ONES28 = sing.tile([NPT, 1], F32)
        nc.vector.memset(ONES28, 1.0)
        BYT = sing.tile([NPT, 1], F32)
        nc.vector.memset(BYT, float(1.0 + _CACHE["c_w0sq"]))
        BM1 = sing.tile([NPT, 1], F32)
        nc.vector.memset(BM1, -1.0)
        BEPSV = sing.tile([1, 1], F32)
        nc.vector.memset(BEPSV, 1e-5)
        MUHSB = sing.tile([128, 1], F16)       # [mu_s;0] fp16, set after AR1
        nc.vector.memset(MUHSB, 0.0)

        YCMB = sing.tile([128, NPX], F16)      # rows 0..126 y', row 127 T^2-1
        MUP = sing.tile([128, NGROUPS], F32)   # per-group per-channel sums
        XP = sing.tile([128, HP, WP], F16)     # [s | s^2] padded image
        nc.vector.memset(XP, 0.0)

        # pixel-scalar fields, [28, 896] (partition = half-band)
        def ps(name, dt=F32):
            t = sing.tile([NPT, PPX], dt, tag=name, name=name)
            return t
        T2M1, TPS, W0DOT, YSQ1, YT = ps("t2m1"), ps("tps"), ps("w0dot"), ps("ysq1"), ps("yt")
        MUDOT, MA, ALPHA, FPS, HPS = ps("mudot"), ps("ma"), ps("alpha"), ps("fps"), ps("hps")
        PSA, PSB, PSC = ps("psa"), ps("psb"), ps("psc")
        TPSB, HPSB, W2B = ps("tpsb", F16), ps("hpsb", F16), ps("w2b", F16)
        YTSQ = ps("ytsq")

        # ================= PHASE 1: conv =================
        with nc.allow_low_precision("fp16 conv by design"):
            # stats matmuls trail one group behind the conv stream so the
            # in-order PE never stalls on the DVE evac / ACT square feeding them
            pend_st = [None]    # (g, cols, ysq) awaiting stats matmuls
            stga = {}           # band -> STGA staging tile

            def flush_stats():
                gp, colsp, ysq_t = pend_st[0]
                pend_st[0] = None
                bandp, kp = divmod(gp, GROUPS_PER_BAND)
                if kp == 0:
                    stga[bandp] = stgp.tile([3, GROUPS_PER_BAND, GROUP_PX],
                                            F32, tag="stg3", name="STGA")
                STGA = stga[bandp]
                psA = pss.tile([3, GROUP_PX], F32, tag="sb")
                nc.tensor.matmul(psA[:], lhsT=SW3[:, 0:3], rhs=YCMB[:, colsp],
                                 start=True, stop=False)
                nc.tensor.matmul(psA[:], lhsT=SW3[:, 3:6], rhs=ysq_t[:],
                                 start=False, stop=True)
                if kp % 2 == 0:
                    nc.vector.tensor_copy(out=STGA[:, kp, :], in_=psA[:])
                else:
                    nc.scalar.activation(out=STGA[:, kp, :], in_=psA[:],
                                         func=AF.Copy)
                if kp == GROUPS_PER_BAND - 1:
                    pslp = slice(2 * bandp, 2 * bandp + 2)
                    nc.sync.dma_start(out=W0DOT[pslp, :], in_=STGA[0:1, :, :])
                    nc.sync.dma_start(out=T2M1[pslp, :], in_=STGA[1:2, :, :])
                    nc.sync.dma_start(out=YSQ1[pslp, :], in_=STGA[2:3, :, :])
                    del stga[bandp]

            for b in range(B_LOC):
                # build XP = [s | s^2] fp16 with padding
                for q in range(CHUNKS_PER_IMG):
                    r0 = q * CHUNK_ROWS
                    stg = stagep.tile([128, CHUNK_ROWS, W], F32, tag="stg")
                    src = x_in[b, 1:CIN, r0:r0 + CHUNK_ROWS, :].rearrange(
                        "c h w -> c (h w)")
                    nc.gpsimd.dma_start(
                        out=stg[0:S].rearrange("c h w -> c (h w)"), in_=src)
                    nc.gpsimd.dma_start(
                        out=stg[64:64 + S].rearrange("c h w -> c (h w)"), in_=src)
                    nc.vector.tensor_scalar_add(
                        XP[0:S, 1 + r0:1 + r0 + CHUNK_ROWS, 1:1 + W],
                        stg[0:S], 0.0)
                    nc.scalar.activation(
                        out=XP[64:64 + S, 1 + r0:1 + r0 + CHUNK_ROWS, 1:1 + W],
                        in_=stg[64:64 + S], func=AF.Square)

                for rb in range(BANDS_PER_IMG):
                    band = b * BANDS_PER_IMG + rb
                    for k in range(GROUPS_PER_BAND):
                        g = band * GROUPS_PER_BAND + k
                        cols = bass.ts(g, GROUP_PX)
                        h0 = rb * BAND_ROWS + k * ROWS_PER_GROUP
                        psum = psy.tile([128, GROUP_PX], F32, tag="psy")
                        for wi in range(9):
                            i, j = divmod(wi, 3)
                            rhs = XP[:, h0 + i:h0 + i + ROWS_PER_GROUP, j:j + W]
                            nc.tensor.matmul(psum[:], lhsT=W9B[:, wi, :], rhs=rhs,
                                             start=(wi == 0), stop=(wi == 8))
                        # evacuate to fp16 + per-channel partial sums (for mu)
                        nc.vector.tensor_scalar(out=YCMB[:, cols], in0=psum[:],
                                                scalar1=0.0, scalar2=None, op0=OP.add,
                                                op1=OP.add, accum_out=MUP[:, g:g + 1])
                        ysq = ysqp.tile([128, GROUP_PX], F16, tag="ysq")
                        nc.scalar.activation(out=ysq, in_=psum[:], func=AF.Square,
                                             scale=0.0625)
                        if pend_st[0] is not None:
                            flush_stats()
                        pend_st[0] = (g, cols, ysq)
            if pend_st[0] is not None:
                flush_stats()

        # ---- pixel-scalar chain, phase 1 ----
        # T = sqrt(1 + T2m1)
        nc.scalar.activation(out=TPS, in_=T2M1, func=AF.Sqrt, bias=1.0)
        # ysqf = ysq1 + 2*T*w0dot + T2m1*c_w0sq ; y_t = sqrt(1 + c_w0sq + ysqf')
        nc.vector.tensor_mul(PSA, TPS, W0DOT)
        nc.vector.scalar_tensor_tensor(out=PSB, in0=PSA, scalar=2.0, in1=YSQ1,
                                       op0=OP.mult, op1=OP.add)
        nc.vector.scalar_tensor_tensor(out=PSC, in0=T2M1, scalar=_CACHE["c_w0sq"],
                                       in1=PSB, op0=OP.mult, op1=OP.add)
        nc.scalar.activation(out=YT, in_=PSC, func=AF.Sqrt, bias=BYT[:])
        nc.vector.tensor_mul(YTSQ, YT, YT)
        # reduced sums for the collective
        MUS = sing.tile([128, 1], F32)
        nc.vector.tensor_reduce(MUS, MUP, axis=AX.X, op=OP.add)
        SR = sing.tile([NPT, 2], F32)
        nc.vector.tensor_reduce(SR[:, 0:1], TPS, axis=AX.X, op=OP.add)
        nc.vector.tensor_reduce(SR[:, 1:2], YT, axis=AX.X, op=OP.add)
        pt = psb.tile([1, GROUP_PX], F32, tag="sb")
        nc.tensor.matmul(pt[0:1, 0:2], lhsT=ONES28, rhs=SR[:], start=True, stop=True)
        SC0 = sing.tile([1, 2], F32)
        nc.vector.tensor_copy(out=SC0, in_=pt[0:1, 0:2])
        nc.sync.dma_start(out=cc1_in[0:128], in_=MUS)
        nc.sync.dma_start(out=cc1_in[128:130], in_=SC0)
        nc.gpsimd.collective_compute("AllReduce", OP.add, replica_groups=groups_all,
                                     ins=[cc1_in[:]], outs=[cc1_out[:]])
        MUSG = sing.tile([128, 1], F32)
        nc.sync.dma_start(out=MUSG, in_=cc1_out[0:128].rearrange("(p o) -> p o", o=1))
        MUSR = sing.tile([1, 130], F32)
        nc.sync.dma_start(out=MUSR, in_=cc1_out[:].rearrange("(o c) -> o c", o=1))
        # warm the PE as soon as the collective lands (WG depends on MUSG)
        with nc.allow_low_precision("warmup"):
            WG = ysqp.tile([128, GROUP_PX], F16, tag="ysq")
            nc.vector.tensor_scalar(out=WG, in0=YCMB[:, 0:GROUP_PX],
                                    scalar1=MUSG[:, 0:1], scalar2=None, op0=OP.mult)
            for _ in range(20):
                wps = psy.tile([128, GROUP_PX], F32, tag="psy")
                nc.tensor.matmul(wps[:], lhsT=IDENT, rhs=WG, start=True, stop=True)

        # ---- mu normalization: row-major on partition 0, all on DVE ----
        invN = 1.0 / float(NPX_GLOB)
        MROW = sing.tile([1, 136], F32)
        # muus row = invN * (sumT * W0 + musg)
        nc.vector.scalar_tensor_tensor(out=MROW[:, 0:M], in0=W0ROW[:, 0:M],
                                       scalar=MUSR[:, 128:129], in1=MUSR[:, 0:M],
                                       op0=OP.mult, op1=OP.add)
        nc.vector.tensor_scalar_mul(MROW[:, 0:M], MROW[:, 0:M], invN)
        nc.vector.tensor_scalar_mul(MROW[:, 128:129], MUSR[:, 129:130], invN)  # mu0u
        SQR = sing.tile([1, 136], F32)
        nc.vector.tensor_mul(SQR[:, 0:M], MROW[:, 0:M], MROW[:, 0:M])
        nc.vector.tensor_reduce(SQR[:, 128:129], SQR[:, 0:M], axis=AX.X, op=OP.add)
        nc.vector.tensor_mul(SQR[:, 129:130], MROW[:, 128:129], MROW[:, 128:129])
        nc.vector.tensor_sub(SQR[:, 129:130], SQR[:, 129:130], SQR[:, 128:129])
        nc.scalar.activation(out=SQR[:, 130:131], in_=SQR[:, 129:130], func=AF.Sqrt)
        nc.vector.reciprocal_approx_fast(out=SQR[:, 131:132], in_=SQR[:, 130:131])
        RN = SQR[:, 131:132]                                    # 1/nrm
        nc.vector.tensor_scalar(out=MROW[:, 0:M], in0=MROW[:, 0:M], scalar1=RN,
                                scalar2=None, op0=OP.mult)      # mu_s row
        nc.vector.tensor_mul(MROW[:, 128:129], MROW[:, 128:129], RN)  # mu0
        # c_muW0 = sum(mu_s * W0)
        nc.vector.tensor_mul(SQR[:, 0:M], MROW[:, 0:M], W0ROW[:, 0:M])
        nc.vector.tensor_reduce(MROW[:, 130:131], SQR[:, 0:M], axis=AX.X, op=OP.add)
        # inv1p = 1/(1+mu0); m0sq1 = mu0^2-1
        nc.vector.tensor_scalar_add(MROW[:, 129:130], MROW[:, 128:129], 1.0)
        nc.vector.reciprocal_approx_fast(out=SQR[:, 132:133], in_=MROW[:, 129:130])
        nc.vector.tensor_copy(out=MROW[:, 129:130], in_=SQR[:, 132:133])
        nc.vector.tensor_mul(MROW[:, 131:132], MROW[:, 128:129], MROW[:, 128:129])
        nc.vector.tensor_scalar_add(MROW[:, 131:132], MROW[:, 131:132], -1.0)
        # SCROW layout: {mu0, inv1p, c_muW0, mu0^2-1} = MROW[128:132]
        SC28 = sing.tile([NPT, 4], F32)
        nc.gpsimd.partition_broadcast(SC28, MROW[:, 128:132])
        with nc.allow_low_precision("fp16 mu by design"):
            # LR1B row0 = -mu_s (same partition: plain DVE write, no DMA)
            nc.vector.tensor_scalar_mul(LR1B[0:1, 0:M], MROW[:, 0:M], -1.0)
            # MUHSB column = musg_col*(invN/nrm) + W0col*(sumT*invN/nrm)
            nc.vector.tensor_scalar_mul(SQR[:, 133:134], RN, invN)
            nc.vector.tensor_mul(SQR[:, 134:135], SQR[:, 133:134], MUSR[:, 128:129])
            PB2 = sing.tile([M, 2], F32)
            nc.gpsimd.partition_broadcast(PB2, SQR[:, 133:135])
            MUHC = sing.tile([M, 1], F32)
            nc.vector.tensor_scalar(out=MUHC, in0=MUSG[0:M, :], scalar1=PB2[:, 0:1],
                                    scalar2=None, op0=OP.mult)
            nc.vector.scalar_tensor_tensor(out=MUHSB[0:M], in0=REDW[:, 0:1],
                                           scalar=PB2[:, 1:2], in1=MUHC,
                                           op0=OP.mult, op1=OP.add)

        if _CACHE.get("debug"):
            nc.sync.dma_start(out=dbg_ycm[:], in_=YCMB[:])
            for i_, t_ in enumerate([T2M1, TPS, W0DOT, YSQ1, YT]):
                nc.sync.dma_start(out=dbg_ps1[i_], in_=t_[:])
            nc.sync.dma_start(out=dbg_mu[0:128], in_=MUSG[:])
            nc.sync.dma_start(out=dbg_mu[128:130], in_=MUSR[:, 128:130])

        # ================= PHASE 2 =================
        with nc.allow_low_precision("fp16 phase2 by design"):
            for band in range(NBANDS):
                psl = slice(2 * band, 2 * band + 2)
                STGC = stgp.tile([3, GROUPS_PER_BAND, GROUP_PX], F32, tag="stg3")
                for k in range(GROUPS_PER_BAND):
                    g = band * GROUPS_PER_BAND + k
                    cols = bass.ts(g, GROUP_PX)
                    psm = psb.tile([1, GROUP_PX], F32, tag="sb")
                    nc.tensor.matmul(psm[:], lhsT=MUHSB, rhs=YCMB[:, cols],
                                     start=True, stop=True)
                    if k % 2 == 0:
                        nc.vector.tensor_copy(out=STGC[0:1, k, :], in_=psm[:])
                    else:
                        nc.scalar.activation(out=STGC[0:1, k, :], in_=psm[:],
                                             func=AF.Copy)
                nc.sync.dma_start(out=MUDOT[psl, :], in_=STGC[0:1, :, :])

            # alpha = clip(mu0*yt - (mudot + T*c_muW0), 1+eps)
            nc.vector.scalar_tensor_tensor(out=MA, in0=TPS, scalar=SC28[:, 2:3],
                                           in1=MUDOT, op0=OP.mult, op1=OP.add)
            nc.vector.scalar_tensor_tensor(out=ALPHA, in0=YT, scalar=SC28[:, 0:1],
                                           in1=MA, op0=OP.mult, op1=OP.subtract)
            nc.vector.tensor_scalar_max(ALPHA, ALPHA, 1.0 + EPS)
            # f = ln(alpha + sqrt(alpha^2-1)) / sqrt(alpha^2-1)   (on DVE+ACT)
            # H = alpha + (yt - alpha*mu0) * inv1p                (on GPSIMD)
            nc.vector.tensor_scalar(out=PSC, in0=ALPHA, scalar1=SC28[:, 0:1],
                                    scalar2=None, op0=OP.mult)
            nc.vector.tensor_mul(PSA, ALPHA, ALPHA)
            nc.scalar.activation(out=PSB, in_=PSA, func=AF.Sqrt, bias=BM1[:])
            nc.gpsimd.tensor_sub(PSC, YT, PSC)
            nc.vector.scalar_tensor_tensor(out=HPS, in0=PSC, scalar=SC28[:, 1:2],
                                           in1=ALPHA, op0=OP.mult, op1=OP.add)
            nc.vector.tensor_add(PSA, ALPHA, PSB)
            nc.scalar.activation(out=PSA, in_=PSA, func=AF.Ln)
            nc.vector.reciprocal_approx_fast(out=FPS, in_=PSB)
            nc.vector.tensor_mul(FPS, FPS, PSA)
            nc.gpsimd.tensor_copy(out=HPSB, in_=HPS)
            nc.scalar.activation(out=TPSB, in_=TPS, func=AF.Copy)
            STSQ = ALPHA    # alpha dead after HPS; reuse its tile
            # stsq = YT^2 - 1 + H*(H*(mu0^2-1) - 2*MA)
            nc.vector.tensor_scalar(out=PSC, in0=HPS, scalar1=SC28[:, 3:4],
                                    scalar2=None, op0=OP.mult)
            nc.vector.scalar_tensor_tensor(out=PSC, in0=MA, scalar=-2.0,
                                           in1=PSC, op0=OP.mult, op1=OP.add)
            nc.gpsimd.tensor_mul(PSC, PSC, HPS)
            nc.vector.tensor_add(STSQ, PSC, YTSQ)
            nc.vector.tensor_scalar_add(STSQ, STSQ, -1.0)

        if _CACHE.get("debug"):
            for i_, t_ in enumerate([MUDOT, FPS, HPS, STSQ]):
                nc.sync.dma_start(out=dbg_ps2[i_], in_=t_[:])

        # var = mean(f^2 * stsq)  -> allreduce
        nc.vector.tensor_mul(PSA, FPS, FPS)
        nc.vector.tensor_mul(PSB, PSA, STSQ)
        VR = sing.tile([NPT, 1], F32)
        nc.vector.tensor_reduce(VR, PSB, axis=AX.X, op=OP.add)
        pt4 = psb.tile([1, GROUP_PX], F32, tag="sb")
        nc.tensor.matmul(pt4[0:1, 0:1], lhsT=ONES28, rhs=VR[:], start=True, stop=True)
        VSC = sing.tile([1, 2], F32)
        nc.vector.tensor_copy(out=VSC[:, 0:1], in_=pt4[0:1, 0:1])
        nc.vector.tensor_copy(out=VSC[:, 1:2], in_=pt4[0:1, 0:1])
        nc.sync.dma_start(out=cc2_in[:], in_=VSC)
        nc.gpsimd.collective_compute("AllReduce", OP.add, replica_groups=groups_all,
                                     ins=[cc2_in[:]], outs=[cc2_out[:]])
        # w2 = gf*sinh(vn)/vn with vn = gf*u, u = sqrt(stsq)  =>  w2 = sinh(vn)/u.
        # u, 0.5/u, f*u are g-independent: compute them during the AR2 wait.
        U = W0DOT
        INVU = YSQ1
        FUU = PSA
        nc.vector.tensor_scalar_max(PSC, STSQ, 1e-8)
        nc.scalar.activation(out=U, in_=PSC, func=AF.Sqrt)
        nc.vector.reciprocal_approx_fast(out=INVU, in_=U)
        nc.vector.tensor_scalar_mul(INVU, INVU, 0.5)
        nc.vector.tensor_mul(FUU, FPS, U)
        VG = sing.tile([1, 2], F32)
        nc.sync.dma_start(out=VG, in_=cc2_out[:].rearrange("(o c) -> o c", o=1))
        GSC = sing.tile([1, 1], F32)
        nc.vector.tensor_scalar_mul(GSC, VG[0:1, 0:1], invN)
        nc.scalar.activation(out=GSC, in_=GSC, func=AF.Sqrt, bias=BEPSV[:])
        nc.vector.reciprocal(GSC, GSC)
        nc.vector.tensor_mul(GSC, GSC, GAM)
        G28 = sing.tile([NPT, 1], F32)
        nc.gpsimd.partition_broadcast(G28, GSC)
        # warm the PE as soon as AR2 lands (WG2 depends on G28)
        with nc.allow_low_precision("warmup"):
            WG2 = ysqp.tile([128, GROUP_PX], F16, tag="ysq")
            nc.vector.tensor_scalar(out=WG2[0:NPT], in0=FPS[:, 0:GROUP_PX],
                                    scalar1=G28[:, 0:1], scalar2=None, op0=OP.mult)
            for _ in range(14):
                wps = psy.tile([128, GROUP_PX], F32, tag="psy")
                nc.tensor.matmul(wps[:], lhsT=IDENT[0:NPT, :], rhs=WG2[0:NPT],
                                 start=True, stop=True)

        # ================= PHASE 3 =================
        # vn = g*(f*u); w2 = sinh(vn)/u = (exp(vn)-exp(-vn)) * (0.5/u)
        VN = TPS
        nc.vector.tensor_scalar(out=VN, in0=FUU, scalar1=G28[:, 0:1],
                                scalar2=None, op0=OP.mult)
        EXT = STSQ
        nc.scalar.activation(out=EXT, in_=VN, func=AF.Exp)
        nc.scalar.activation(out=PSC, in_=VN, func=AF.Exp, scale=-1.0)
        nc.vector.tensor_sub(EXT, EXT, PSC)                         # 2*sinh
        W2 = MUDOT
        nc.vector.tensor_mul(W2, EXT, INVU)
        with nc.allow_low_precision("fp16 w2 by design"):
            nc.scalar.activation(out=W2B, in_=W2, func=AF.Copy)
        if _CACHE.get("debug"):
            nc.sync.dma_start(out=dbg_ps2[5], in_=W2[:])

        out_flat = [out_d[b_, 1:COUT].rearrange("c h w -> c (h w)")
                    for b_ in range(B_LOC)]

        RSQ2 = MA    # mu-dot accumulator tile is dead after stsq; reuse

        def stage_band(band):
            psl = slice(2 * band, 2 * band + 2)
            HT = stgp.tile([2, GROUPS_PER_BAND, GROUP_PX], F16, tag="htb")
            nc.sync.dma_start(out=HT[0:1, :, :], in_=HPSB[psl, :])
            nc.sync.dma_start(out=HT[1:2, :, :], in_=TPSB[psl, :])
            W2S = stgp.tile([1, GROUPS_PER_BAND, GROUP_PX], F16, tag="w2s")
            nc.sync.dma_start(out=W2S[0:1, :, :], in_=W2B[psl, :])
            W2R = w2rp.tile([128, GROUPS_PER_BAND, GROUP_PX], F16, tag="w2r")
            nc.gpsimd.partition_broadcast(W2R, W2S[0:1, :, :])
            return HT, W2R

        with nc.allow_low_precision("fp16 phase3 by design"):
            nxt = stage_band(0)
            OUTCH = None
            for band in range(NBANDS):
                b, rb = divmod(band, BANDS_PER_IMG)
                psl = slice(2 * band, 2 * band + 2)
                HT, W2R = nxt
                if band + 1 < NBANDS:
                    nxt = stage_band(band + 1)
                ch = rb % OUT_BANDS          # position within output chunk
                if ch == 0:
                    OUTCH = outp.tile([128, OUT_COLS], F32, tag="outch")
                STGE = stgp.tile([3, GROUPS_PER_BAND, GROUP_PX], F32, tag="stg3")
                for blk in range(2):
                    pr2 = pr2p.tile([128, 2, 512], F32, tag="pr2")
                    for j in range(2):
                        k = blk * 2 + j
                        g = band * GROUPS_PER_BAND + k
                        cols = bass.ts(g, GROUP_PX)
                        nc.tensor.matmul(pr2[:, j, 0:GROUP_PX], lhsT=LR1B,
                                         rhs=HT[:, k, :], start=True, stop=False)
                        nc.tensor.matmul(pr2[:, j, 0:GROUP_PX], lhsT=IDENT,
                                         rhs=YCMB[:, cols], start=False, stop=True)
                    bsl = slice((ch * 4 + blk * 2) * GROUP_PX,
                                (ch * 4 + blk * 2 + 2) * GROUP_PX)
                    nc.vector.scalar_tensor_tensor(
                        out=OUTCH[:, bsl].rearrange("p (b c) -> p b c", b=2),
                        in0=pr2[:, :, 0:GROUP_PX], scalar=0.0,
                        in1=W2R[:, blk * 2:blk * 2 + 2, :],
                        op0=OP.max, op1=OP.mult)
                    sqo = ysqp.tile([128, 2 * GROUP_PX], F16, tag="ysq")
                    nc.scalar.activation(out=sqo, in_=OUTCH[:, bsl],
                                         func=AF.Square, scale=0.0625)
                    for j in range(2):
                        k = blk * 2 + j
                        psr = psb.tile([1, GROUP_PX], F32, tag="sb")
                        nc.tensor.matmul(psr[:], lhsT=O256Z,
                                         rhs=sqo[:, bass.ts(j, GROUP_PX)],
                                         start=True, stop=True)
                        if k % 2 == 0:
                            nc.scalar.activation(out=STGE[0:1, k, :], in_=psr[:],
                                                 func=AF.Copy)
                        else:
                            nc.vector.tensor_copy(out=STGE[0:1, k, :], in_=psr[:])
                nc.sync.dma_start(out=RSQ2[psl, :], in_=STGE[0:1, :, :])
                if ch == OUT_BANDS - 1 or rb == BANDS_PER_IMG - 1:
                    ncols = (ch + 1) * BAND_ROWS * W
                    col0 = (rb - ch) * BAND_ROWS * W
                    dst = out_flat[b][:, col0:col0 + ncols]
                    for i in range(8):
                        p0 = i * 16
                        p1 = min(M, p0 + 16)
                        eng = nc.gpsimd if i < 4 else (nc.sync if i < 6 else nc.scalar)
                        eng.dma_start(out=dst[p0:p1, :],
                                      in_=OUTCH[p0:p1, 0:ncols])

        # rt = sqrt(1 + sum rs^2) -> channel 0 plane
        RT = T2M1
        nc.scalar.activation(out=RT, in_=RSQ2, func=AF.Sqrt, bias=1.0)
        nc.gpsimd.dma_start(out=out_d[:, 0, :, :], in_=RT)

    nc.compile()
    return nc


def _prep_consts(W):
    W = np.asarray(W, np.float32)
    f16 = np.float16
    w9 = np.zeros((128, 9, 128), np.float32)
    for wi in range(9):
        w9[0:S, wi, 0:M] = W[:, 1 + wi * S:1 + (wi + 1) * S].T
        w9[64:64 + S, wi, 127] = 1.0
    sw3 = np.zeros((128, 8), np.float32)
    sw3[0:M, 0] = W[:, 0]
    sw3[127, 1] = 1.0
    sw3[0:M, 5] = 256.0
    sw3[0:M, 6] = 1.0
    sw3[0:M, 7] = 256.0
    redw = np.zeros((M, 3), np.float32)
    redw[:, 0] = W[:, 0]
    redw[:, 2] = 1.0
    lr1 = np.zeros((2, 128), np.float32)
    lr1[1, 0:M] = W[:, 0]
    ident = np.eye(128, dtype=np.float32)
    w0row = np.zeros((1, 128), np.float32)
    w0row[0, 0:M] = W[:, 0]
    c_w0sq = float(np.float32((W[:, 0].astype(np.float64) ** 2).sum()))
    return (w9.reshape(128, 9 * 128).astype(f16), sw3.astype(f16), redw, w0row,
            lr1.astype(f16), ident.astype(f16), c_w0sq)


def _in_maps(x, W, gamma):
    x = np.ascontiguousarray(np.asarray(x, np.float32))
    gamma = np.asarray(gamma, np.float32)
    w9, sw3, redw, w0row, lr1, ident, c_w0sq = _prep_consts(W)
    if "nc" not in _CACHE:
        _CACHE["c_w0sq"] = c_w0sq
        _CACHE["nc"] = _build_nc()
    maps = []
    for c in range(NCORES):
        maps.append({
            "x": x[c * B_LOC:(c + 1) * B_LOC],
            "w9": w9, "sw3": sw3, "redw": redw, "w0row": w0row,
            "lr1i": lr1, "identi": ident,
            "gamma": gamma,
        })
    return _CACHE["nc"], maps


def kernel(x, W, gamma, beta):
    beta = np.asarray(beta, np.float32)
    gamma = np.asarray(gamma, np.float32)
    assert abs(float(beta[0]) - 1.0) < 1e-6 and np.all(np.abs(beta[1:]) < 1e-6), \
        "kernel specialized for beta == Lorentz origin"
    assert float(gamma[0]) > 0.0
    nc, in_maps = _in_maps(x, W, gamma)
    res = run_bass_kernel_spmd(nc, in_maps, list(range(NCORES)))
    out = np.concatenate([res.results[c]["out"] for c in range(NCORES)], axis=0)
    return out


def run_traced(inputs, tmpdir=None):
    """Run with NTFF tracing; returns (exec_time_ns, BassKernelResults)."""
    nc, in_maps = _in_maps(inputs["x"], inputs["W"], inputs["gamma"])
    res = run_bass_kernel_spmd(nc, in_maps, list(range(NCORES)),
                               trace=True, tmpdir=tmpdir)
    return res.exec_time_ns, res


def simulate(inputs, debug=True):
    """Run the kernel through MultiCoreSim; returns list of per-core output dicts."""
    from concourse.bass_interp import MultiCoreSim
    _CACHE.clear()
    _CACHE["debug"] = debug
    x = np.asarray(inputs["x"], np.float32)
    w9, sw3, redw, w0row, lr1, ident, c_w0sq = _prep_consts(inputs["W"])
    _CACHE["c_w0sq"] = c_w0sq
    nc = _build_nc()
    sim = MultiCoreSim(nc, num_cores=NCORES)
    for c in range(NCORES):
        cs = sim.cores[c]
        cs.tensor("x")[:] = x[c * B_LOC:(c + 1) * B_LOC]
        cs.tensor("w9")[:] = w9
        cs.tensor("sw3")[:] = sw3
        cs.tensor("redw")[:] = redw
        cs.tensor("w0row")[:] = w0row
        cs.tensor("lr1i")[:] = lr1
        cs.tensor("identi")[:] = ident
        cs.tensor("gamma")[:] = np.asarray(inputs["gamma"], np.float32)
    sim.simulate(check_with_hw=False)
    names = ["out"]
    if debug:
        names += ["dbg_ycm", "dbg_ps1", "dbg_ps2", "dbg_mu"]
    return [{n: np.array(sim.cores[c].tensor(n)) for n in names}
            for c in range(NCORES)]


if __name__ == "__main__":
    rng = np.random.default_rng(0)
    x = rng.standard_normal((B_GLOB, CIN, H, W), dtype=np.float32)
    W_ = (rng.standard_normal((M, D), dtype=np.float32) / np.sqrt(D)).astype(np.float32)
    gamma = np.ones((1,), np.float32)
    beta = np.zeros((COUT,), np.float32); beta[0] = 1.0
    out = kernel(x=x, W=W_, gamma=gamma, beta=beta)
    print("out", out.shape, out.dtype, np.abs(out).max())


# revision 74
# speedup vs baseline: 1.0365x; 1.0225x over previous
"""Trainium2 Bass kernel for nn_EuclideanToLorentzConv (8-core data-parallel).

v5 — fp16 matmul path, SWDGE bulk DMA, algebraic stsq, [28,896] pixel layout:
  * Conv as 9 window-matmuls in fp16 reading a padded SBUF-resident
    [128,114,114] image XP = [s | s^2] built once per image via DVE/ACT casts.
  * Bulk HBM traffic (x loads, output chunks) via gpsimd SWDGE dma_start
    (spreads across 16 SDMA engines); sync-ring HWDGE only carries small
    reshape DMAs (all HWDGE descriptors execute on SDMA engine 0).
  * Pixel-scalar fields live as [28, 896] (partition = half-band) so every
    pixel<->channel reshape DMA is 1-2 descriptors instead of 4.
  * Per-pixel stats via K=128 PE matmuls accumulated into one [3,448]
    PSUM tile per group; stsq has a closed form (no phase-2 tmp pass);
    tmp recomputed in fp32 psum in phase 3 via an identity matmul.
  * w2 per-pixel scale broadcast via K=1 PE matmul + copy to SBUF.
"""

import sys
import numpy as np
from contextlib import ExitStack

sys.path.insert(0, "/opt/trn_rl_repo")

import concourse.bass as bass  # noqa: E402
import concourse.tile as tile  # noqa: E402
from concourse import mybir, bacc  # noqa: E402
from concourse.bass_utils import run_bass_kernel_spmd  # noqa: E402

F32 = mybir.dt.float32
F16 = mybir.dt.float16
AX = mybir.AxisListType
OP = mybir.AluOpType
AF = mybir.ActivationFunctionType

# ---- problem constants (hardcoded; kernel.py must be self-contained) ----
NCORES = 8
B_GLOB, CIN, H, W = 16, 64, 112, 112
B_LOC = B_GLOB // NCORES            # 2 images per core
S = CIN - 1                         # 63 space channels in
M = 127                             # space channels out
COUT = M + 1
D = 9 * S + 1                       # 568
EPS = 1e-6

HP, WP = H + 2, W + 2               # padded 114x114
ROWS_PER_GROUP = 4
GROUP_PX = ROWS_PER_GROUP * W       # 448
BAND_ROWS = 16                      # output rows per band
GROUPS_PER_BAND = BAND_ROWS // ROWS_PER_GROUP   # 4
BANDS_PER_IMG = H // BAND_ROWS      # 7
NBANDS = B_LOC * BANDS_PER_IMG      # 14
NGROUPS = NBANDS * GROUPS_PER_BAND  # 56
NPX = NGROUPS * GROUP_PX            # 25088 pixels per core
NPX_GLOB = B_GLOB * H * W           # 200704

NPT = 2 * NBANDS                    # 28 pixel-tile partitions (half-band each)
PPX = NPX // NPT                    # 896 pixels per partition

CHUNK_ROWS = 7                      # x staging chunk (rows per chunk)
CHUNKS_PER_IMG = H // CHUNK_ROWS    # 8
OUT_BANDS = 2                       # output chunk = 2 bands -> 896B SWDGE descs
OUT_COLS = OUT_BANDS * BAND_ROWS * W  # 3584

_CACHE = {}


def _build_nc():
    nc = bacc.Bacc("TRN2", target_bir_lowering=False, debug=False,
                   num_devices=NCORES)

    x_in = nc.dram_tensor("x", [B_LOC, CIN, H, W], F32, kind="ExternalInput")
    w9_in = nc.dram_tensor("w9", [128, 9 * 128], F16, kind="ExternalInput")
    sw3_in = nc.dram_tensor("sw3", [128, 8], F16, kind="ExternalInput")
    redw_in = nc.dram_tensor("redw", [M, 3], F32, kind="ExternalInput")
    w0row_in = nc.dram_tensor("w0row", [1, 128], F32, kind="ExternalInput")
    lr1_in = nc.dram_tensor("lr1i", [2, 128], F16, kind="ExternalInput")
    ident_in = nc.dram_tensor("identi", [128, 128], F16, kind="ExternalInput")
    gamma_in = nc.dram_tensor("gamma", [1], F32, kind="ExternalInput")
    out_d = nc.dram_tensor("out", [B_LOC, COUT, H, W], F32,
                           kind="ExternalOutput")

    if _CACHE.get("debug"):
        dbg_ycm = nc.dram_tensor("dbg_ycm", [128, NPX], F16, kind="ExternalOutput")
        dbg_ps1 = nc.dram_tensor("dbg_ps1", [5, NPT, PPX], F32, kind="ExternalOutput")
        dbg_ps2 = nc.dram_tensor("dbg_ps2", [6, NPT, PPX], F32, kind="ExternalOutput")
        dbg_mu = nc.dram_tensor("dbg_mu", [130], F32, kind="ExternalOutput")
    cc1_in = nc.dram_tensor("cc1_in", [130], F32)
    cc1_out = nc.dram_tensor("cc1_out", [130], F32, addr_space="Shared")
    cc2_in = nc.dram_tensor("cc2_in", [2], F32)
    cc2_out = nc.dram_tensor("cc2_out", [2], F32, addr_space="Shared")
    groups_all = [list(range(NCORES))]

    with tile.TileContext(nc) as tc, ExitStack() as ctx:
        sing = ctx.enter_context(tc.tile_pool(name="sing", bufs=1))
        ysqp = ctx.enter_context(tc.tile_pool(name="ysq", bufs=2))
        w2rp = ctx.enter_context(tc.tile_pool(name="w2rp", bufs=2))
        outp = ctx.enter_context(tc.tile_pool(name="outp", bufs=2))
        stgp = ctx.enter_context(tc.tile_pool(name="stg", bufs=2))
        stagep = ctx.enter_context(tc.tile_pool(name="stage", bufs=2))
        psy = ctx.enter_context(tc.tile_pool(name="psy", bufs=2, space="PSUM"))
        pss = ctx.enter_context(tc.tile_pool(name="pss", bufs=2, space="PSUM"))
        psb = pss
        pr2p = ctx.enter_context(tc.tile_pool(name="pr2", bufs=2, space="PSUM"))

        # ---- static SBUF ----
        W9B = sing.tile([128, 9, 128], F16)
        nc.sync.dma_start(out=W9B, in_=w9_in[:].rearrange("p (w m) -> p w m", w=9))
        # SW3 stat weights [128, 8]:
        #   col0 = [W0;0]   col1 = e127      col2 = 0    (applied to y')
        #   col3 = 0        col4 = 0         col5 = 256*[1;..;1;0]  ((y'/16)^2)
        #   col6 = [1;...;1;0] (O127Z)       col7 = 256*[1;..;1;0]
        SW3 = sing.tile([128, 8], F16)
        nc.sync.dma_start(out=SW3, in_=sw3_in[:])
        O127Z = SW3[:, 6:7]
        O256Z = SW3[:, 7:8]
        REDW = sing.tile([M, 3], F32)
        nc.sync.dma_start(out=REDW, in_=redw_in[:])
        W0ROW = sing.tile([1, 128], F32)
        nc.sync.dma_start(out=W0ROW, in_=w0row_in[:])
        LR1B = sing.tile([2, 128], F16)
        nc.sync.dma_start(out=LR1B, in_=lr1_in[:])
        IDENT = sing.tile([128, 128], F16)
        nc.sync.dma_start(out=IDENT, in_=ident_in[:])
        GAM = sing.tile([1, 1], F32)
        nc.sync.dma_start(out=GAM, in_=gamma_in[:].rearrange("(o c) -> o c", o=1))
        ONES28 = sing.tile([NPT, 1], F32)
        nc.vector.memset(ONES28, 1.0)
        BYT = sing.tile([NPT, 1], F32)
        nc.vector.memset(BYT, float(1.0 + _CACHE["c_w0sq"]))
        BM1 = sing.tile([NPT, 1], F32)
        nc.vector.memset(BM1, -1.0)
        BEPSV = sing.tile([1, 1], F32)
        nc.vector.memset(BEPSV, 1e-5)
        MUHSB = sing.tile([128, 1], F16)       # [mu_s;0] fp16, set after AR1
        nc.vector.memset(MUHSB, 0.0)

        YCMB = sing.tile([128, NPX], F16)      # rows 0..126 y', row 127 T^2-1
        MUP = sing.tile([128, NGROUPS], F32)   # per-group per-channel sums
        XP = sing.tile([128, HP, WP], F16)     # [s | s^2] padded image
        nc.vector.memset(XP, 0.0)

        # pixel-scalar fields, [28, 896] (partition = half-band)
        def ps(name, dt=F32):
            t = sing.tile([NPT, PPX], dt, tag=name, name=name)
            return t
        T2M1, TPS, W0DOT, YSQ1, YT = ps("t2m1"), ps("tps"), ps("w0dot"), ps("ysq1"), ps("yt")
        MUDOT, MA, ALPHA, FPS, HPS = ps("mudot"), ps("ma"), ps("alpha"), ps("fps"), ps("hps")
        PSA, PSB, PSC = ps("psa"), ps("psb"), ps("psc")
        TPSB, HPSB, W2B = ps("tpsb", F16), ps("hpsb", F16), ps("w2b", F16)
        YTSQ = ps("ytsq")

        # ================= PHASE 1: conv =================
        with nc.allow_low_precision("fp16 conv by design"):
            # stats matmuls trail one group behind the conv stream so the
            # in-order PE never stalls on the DVE evac / ACT square feeding them
            pend_st = [None]    # (g, cols, ysq) awaiting stats matmuls
            stga = {}           # band -> STGA staging tile

            def flush_stats():
                gp, colsp, ysq_t = pend_st[0]
                pend_st[0] = None
                bandp, kp = divmod(gp, GROUPS_PER_BAND)
                if kp == 0:
                    stga[bandp] = stgp.tile([3, GROUPS_PER_BAND, GROUP_PX],
                                            F32, tag="stg3", name="STGA")
                STGA = stga[bandp]
                psA = pss.tile([3, GROUP_PX], F32, tag="sb")
                nc.tensor.matmul(psA[:], lhsT=SW3[:, 0:3], rhs=YCMB[:, colsp],
                                 start=True, stop=False)
                nc.tensor.matmul(psA[:], lhsT=SW3[:, 3:6], rhs=ysq_t[:],
                                 start=False, stop=True)
                if kp % 2 == 0:
                    nc.vector.tensor_copy(out=STGA[:, kp, :], in_=psA[:])
                else:
                    nc.scalar.activation(out=STGA[:, kp, :], in_=psA[:],
                                         func=AF.Copy)
                if kp == GROUPS_PER_BAND - 1:
                    pslp = slice(2 * bandp, 2 * bandp + 2)
                    nc.sync.dma_start(out=W0DOT[pslp, :], in_=STGA[0:1, :, :])
                    nc.sync.dma_start(out=T2M1[pslp, :], in_=STGA[1:2, :, :])
                    nc.sync.dma_start(out=YSQ1[pslp, :], in_=STGA[2:3, :, :])
                    del stga[bandp]

            for b in range(B_LOC):
                # build XP = [s | s^2] fp16 with padding
                for q in range(CHUNKS_PER_IMG):
                    r0 = q * CHUNK_ROWS
                    stg = stagep.tile([128, CHUNK_ROWS, W], F32, tag="stg")
                    src = x_in[b, 1:CIN, r0:r0 + CHUNK_ROWS, :].rearrange(
                        "c h w -> c (h w)")
                    nc.gpsimd.dma_start(
                        out=stg[0:S].rearrange("c h w -> c (h w)"), in_=src)
                    nc.gpsimd.dma_start(
                        out=stg[64:64 + S].rearrange("c h w -> c (h w)"), in_=src)
                    nc.vector.tensor_scalar_add(
                        XP[0:S, 1 + r0:1 + r0 + CHUNK_ROWS, 1:1 + W],
                        stg[0:S], 0.0)
                    nc.scalar.activation(
                        out=XP[64:64 + S, 1 + r0:1 + r0 + CHUNK_ROWS, 1:1 + W],
                        in_=stg[64:64 + S], func=AF.Square)

                for rb in range(BANDS_PER_IMG):
                    band = b * BANDS_PER_IMG + rb
                    for k in range(GROUPS_PER_BAND):
                        g = band * GROUPS_PER_BAND + k
                        cols = bass.ts(g, GROUP_PX)
                        h0 = rb * BAND_ROWS + k * ROWS_PER_GROUP
                        psum = psy.tile([128, GROUP_PX], F32, tag="psy")
                        for wi in range(9):
                            i, j = divmod(wi, 3)
                            rhs = XP[:, h0 + i:h0 + i + ROWS_PER_GROUP, j:j + W]
                            nc.tensor.matmul(psum[:], lhsT=W9B[:, wi, :], rhs=rhs,
                                             start=(wi == 0), stop=(wi == 8))
                        # evacuate to fp16 + per-channel partial sums (for mu)
                        nc.vector.tensor_scalar(out=YCMB[:, cols], in0=psum[:],
                                                scalar1=0.0, scalar2=None, op0=OP.add,
                                                op1=OP.add, accum_out=MUP[:, g:g + 1])
                        ysq = ysqp.tile([128, GROUP_PX], F16, tag="ysq")
                        nc.scalar.activation(out=ysq, in_=psum[:], func=AF.Square,
                                             scale=0.0625)
                        if pend_st[0] is not None:
                            flush_stats()
                        pend_st[0] = (g, cols, ysq)
            if pend_st[0] is not None:
                flush_stats()

        # ---- pixel-scalar chain, phase 1 ----
        # T = sqrt(1 + T2m1)
        nc.scalar.activation(out=TPS, in_=T2M1, func=AF.Sqrt, bias=1.0)
        # ysqf = ysq1 + 2*T*w0dot + T2m1*c_w0sq ; y_t = sqrt(1 + c_w0sq + ysqf')
        nc.vector.tensor_mul(PSA, TPS, W0DOT)
        nc.vector.scalar_tensor_tensor(out=PSB, in0=PSA, scalar=2.0, in1=YSQ1,
                                       op0=OP.mult, op1=OP.add)
        nc.vector.scalar_tensor_tensor(out=PSC, in0=T2M1, scalar=_CACHE["c_w0sq"],
                                       in1=PSB, op0=OP.mult, op1=OP.add)
        nc.scalar.activation(out=YT, in_=PSC, func=AF.Sqrt, bias=BYT[:])
        nc.vector.tensor_mul(YTSQ, YT, YT)
        # reduced sums for the collective
        MUS = sing.tile([128, 1], F32)
        nc.vector.tensor_reduce(MUS, MUP, axis=AX.X, op=OP.add)
        SR = sing.tile([NPT, 2], F32)
        nc.vector.tensor_reduce(SR[:, 0:1], TPS, axis=AX.X, op=OP.add)
        nc.vector.tensor_reduce(SR[:, 1:2], YT, axis=AX.X, op=OP.add)
        pt = psb.tile([1, GROUP_PX], F32, tag="sb")
        nc.tensor.matmul(pt[0:1, 0:2], lhsT=ONES28, rhs=SR[:], start=True, stop=True)
        SC0 = sing.tile([1, 2], F32)
        nc.vector.tensor_copy(out=SC0, in_=pt[0:1, 0:2])
        nc.sync.dma_start(out=cc1_in[0:128], in_=MUS)
        nc.sync.dma_start(out=cc1_in[128:130], in_=SC0)
        nc.gpsimd.collective_compute("AllReduce", OP.add, replica_groups=groups_all,
                                     ins=[cc1_in[:]], outs=[cc1_out[:]])
        MUSG = sing.tile([128, 1], F32)
        nc.sync.dma_start(out=MUSG, in_=cc1_out[0:128].rearrange("(p o) -> p o", o=1))
        MUSR = sing.tile([1, 130], F32)
        nc.sync.dma_start(out=MUSR, in_=cc1_out[:].rearrange("(o c) -> o c", o=1))
        # warm the PE as soon as the collective lands (WG depends on MUSG)
        with nc.allow_low_precision("warmup"):
            WG = ysqp.tile([128, GROUP_PX], F16, tag="ysq")
            nc.vector.tensor_scalar(out=WG, in0=YCMB[:, 0:GROUP_PX],
                                    scalar1=MUSG[:, 0:1], scalar2=None, op0=OP.mult)
            for _ in range(20):
                wps = psy.tile([128, GROUP_PX], F32, tag="psy")
                nc.tensor.matmul(wps[:], lhsT=IDENT, rhs=WG, start=True, stop=True)

        # ---- mu normalization: row-major on partition 0, all on DVE ----
        invN = 1.0 / float(NPX_GLOB)
        MROW = sing.tile([1, 136], F32)
        # muus row = invN * (sumT * W0 + musg)
        nc.vector.scalar_tensor_tensor(out=MROW[:, 0:M], in0=W0ROW[:, 0:M],
                                       scalar=MUSR[:, 128:129], in1=MUSR[:, 0:M],
                                       op0=OP.mult, op1=OP.add)
        nc.vector.tensor_scalar_mul(MROW[:, 0:M], MROW[:, 0:M], invN)
        nc.vector.tensor_scalar_mul(MROW[:, 128:129], MUSR[:, 129:130], invN)  # mu0u
        SQR = sing.tile([1, 136], F32)
        nc.vector.tensor_mul(SQR[:, 0:M], MROW[:, 0:M], MROW[:, 0:M])
        nc.vector.tensor_reduce(SQR[:, 128:129], SQR[:, 0:M], axis=AX.X, op=OP.add)
        nc.vector.tensor_mul(SQR[:, 129:130], MROW[:, 128:129], MROW[:, 128:129])
        nc.vector.tensor_sub(SQR[:, 129:130], SQR[:, 129:130], SQR[:, 128:129])
        nc.scalar.activation(out=SQR[:, 130:131], in_=SQR[:, 129:130], func=AF.Sqrt)
        nc.vector.reciprocal_approx_fast(out=SQR[:, 131:132], in_=SQR[:, 130:131])
        RN = SQR[:, 131:132]                                    # 1/nrm
        nc.vector.tensor_scalar(out=MROW[:, 0:M], in0=MROW[:, 0:M], scalar1=RN,
                                scalar2=None, op0=OP.mult)      # mu_s row
        nc.vector.tensor_mul(MROW[:, 128:129], MROW[:, 128:129], RN)  # mu0
        # c_muW0 = sum(mu_s * W0)
        nc.vector.tensor_mul(SQR[:, 0:M], MROW[:, 0:M], W0ROW[:, 0:M])
        nc.vector.tensor_reduce(MROW[:, 130:131], SQR[:, 0:M], axis=AX.X, op=OP.add)
        # inv1p = 1/(1+mu0); m0sq1 = mu0^2-1
        nc.vector.tensor_scalar_add(MROW[:, 129:130], MROW[:, 128:129], 1.0)
        nc.vector.reciprocal_approx_fast(out=SQR[:, 132:133], in_=MROW[:, 129:130])
        nc.vector.tensor_copy(out=MROW[:, 129:130], in_=SQR[:, 132:133])
        nc.vector.tensor_mul(MROW[:, 131:132], MROW[:, 128:129], MROW[:, 128:129])
        nc.vector.tensor_scalar_add(MROW[:, 131:132], MROW[:, 131:132], -1.0)
        # SCROW layout: {mu0, inv1p, c_muW0, mu0^2-1} = MROW[128:132]
        SC28 = sing.tile([NPT, 4], F32)
        nc.gpsimd.partition_broadcast(SC28, MROW[:, 128:132])
        with nc.allow_low_precision("fp16 mu by design"):
            # LR1B row0 = -mu_s (same partition: plain DVE write, no DMA)
            nc.vector.tensor_scalar_mul(LR1B[0:1, 0:M], MROW[:, 0:M], -1.0)
            # MUHSB column = musg_col*(invN/nrm) + W0col*(sumT*invN/nrm)
            nc.vector.tensor_scalar_mul(SQR[:, 133:134], RN, invN)
            nc.vector.tensor_mul(SQR[:, 134:135], SQR[:, 133:134], MUSR[:, 128:129])
            PB2 = sing.tile([M, 2], F32)
            nc.gpsimd.partition_broadcast(PB2, SQR[:, 133:135])
            MUHC = sing.tile([M, 1], F32)
            nc.vector.tensor_scalar(out=MUHC, in0=MUSG[0:M, :], scalar1=PB2[:, 0:1],
                                    scalar2=None, op0=OP.mult)
            nc.vector.scalar_tensor_tensor(out=MUHSB[0:M], in0=REDW[:, 0:1],
                                           scalar=PB2[:, 1:2], in1=MUHC,
                                           op0=OP.mult, op1=OP.add)

        if _CACHE.get("debug"):
            nc.sync.dma_start(out=dbg_ycm[:], in_=YCMB[:])
            for i_, t_ in enumerate([T2M1, TPS, W0DOT, YSQ1, YT]):
                nc.sync.dma_start(out=dbg_ps1[i_], in_=t_[:])
            nc.sync.dma_start(out=dbg_mu[0:128], in_=MUSG[:])
            nc.sync.dma_start(out=dbg_mu[128:130], in_=MUSR[:, 128:130])

        # ================= PHASE 2 =================
        with nc.allow_low_precision("fp16 phase2 by design"):
            for band in range(NBANDS):
                psl = slice(2 * band, 2 * band + 2)
                STGC = stgp.tile([3, GROUPS_PER_BAND, GROUP_PX], F32, tag="stg3")
                for k in range(GROUPS_PER_BAND):
                    g = band * GROUPS_PER_BAND + k
                    cols = bass.ts(g, GROUP_PX)
                    psm = psb.tile([1, GROUP_PX], F32, tag="sb")
                    nc.tensor.matmul(psm[:], lhsT=MUHSB, rhs=YCMB[:, cols],
                                     start=True, stop=True)
                    if k % 2 == 0:
                        nc.vector.tensor_copy(out=STGC[0:1, k, :], in_=psm[:])
                    else:
                        nc.scalar.activation(out=STGC[0:1, k, :], in_=psm[:],
                                             func=AF.Copy)
                nc.sync.dma_start(out=MUDOT[psl, :], in_=STGC[0:1, :, :])

            # alpha = clip(mu0*yt - (mudot + T*c_muW0), 1+eps)
            nc.vector.scalar_tensor_tensor(out=MA, in0=TPS, scalar=SC28[:, 2:3],
                                           in1=MUDOT, op0=OP.mult, op1=OP.add)
            nc.vector.scalar_tensor_tensor(out=ALPHA, in0=YT, scalar=SC28[:, 0:1],
                                           in1=MA, op0=OP.mult, op1=OP.subtract)
            nc.vector.tensor_scalar_max(ALPHA, ALPHA, 1.0 + EPS)
            # f = ln(alpha + sqrt(alpha^2-1)) / sqrt(alpha^2-1)   (on DVE+ACT)
            # H = alpha + (yt - alpha*mu0) * inv1p                (on GPSIMD)
            nc.vector.tensor_scalar(out=PSC, in0=ALPHA, scalar1=SC28[:, 0:1],
                                    scalar2=None, op0=OP.mult)
            nc.vector.tensor_mul(PSA, ALPHA, ALPHA)
            nc.scalar.activation(out=PSB, in_=PSA, func=AF.Sqrt, bias=BM1[:])
            nc.gpsimd.tensor_sub(PSC, YT, PSC)
            nc.vector.scalar_tensor_tensor(out=HPS, in0=PSC, scalar=SC28[:, 1:2],
                                           in1=ALPHA, op0=OP.mult, op1=OP.add)
            nc.vector.tensor_add(PSA, ALPHA, PSB)
            nc.scalar.activation(out=PSA, in_=PSA, func=AF.Ln)
            nc.vector.reciprocal_approx_fast(out=FPS, in_=PSB)
            nc.vector.tensor_mul(FPS, FPS, PSA)
            nc.gpsimd.tensor_copy(out=HPSB, in_=HPS)
            nc.scalar.activation(out=TPSB, in_=TPS, func=AF.Copy)
            STSQ = ALPHA    # alpha dead after HPS; reuse its tile
            # stsq = YT^2 - 1 + H*(H*(mu0^2-1) - 2*MA)
            nc.vector.tensor_scalar(out=PSC, in0=HPS, scalar1=SC28[:, 3:4],
                                    scalar2=None, op0=OP.mult)
            nc.vector.scalar_tensor_tensor(out=PSC, in0=MA, scalar=-2.0,
                                           in1=PSC, op0=OP.mult, op1=OP.add)
            nc.gpsimd.tensor_mul(PSC, PSC, HPS)
            nc.vector.tensor_add(STSQ, PSC, YTSQ)
            nc.vector.tensor_scalar_add(STSQ, STSQ, -1.0)

        if _CACHE.get("debug"):
            for i_, t_ in enumerate([MUDOT, FPS, HPS, STSQ]):
                nc.sync.dma_start(out=dbg_ps2[i_], in_=t_[:])

        # var = mean(f^2 * stsq)  -> allreduce
        nc.vector.tensor_mul(PSA, FPS, FPS)
        nc.vector.tensor_mul(PSB, PSA, STSQ)
        VR = sing.tile([NPT, 1], F32)
        nc.vector.tensor_reduce(VR, PSB, axis=AX.X, op=OP.add)
        pt4 = psb.tile([1, GROUP_PX], F32, tag="sb")
        nc.tensor.matmul(pt4[0:1, 0:1], lhsT=ONES28, rhs=VR[:], start=True, stop=True)
        VSC = sing.tile([1, 2], F32)
        nc.vector.tensor_copy(out=VSC[:, 0:1], in_=pt4[0:1, 0:1])
        nc.vector.tensor_copy(out=VSC[:, 1:2], in_=pt4[0:1, 0:1])
        nc.sync.dma_start(out=cc2_in[:], in_=VSC)
        nc.gpsimd.collective_compute("AllReduce", OP.add, replica_groups=groups_all,
                                     ins=[cc2_in[:]], outs=[cc2_out[:]])
        # w2 = gf*sinh(vn)/vn with vn = gf*u, u = sqrt(stsq)  =>  w2 = sinh(vn)/u.
        # u, 0.5/u, f*u are g-independent: compute them during the AR2 wait.
        U = W0DOT
        INVU = YSQ1
        FUU = PSA
        nc.vector.tensor_scalar_max(PSC, STSQ, 1e-8)
        nc.scalar.activation(out=U, in_=PSC, func=AF.Sqrt)
        nc.vector.reciprocal_approx_fast(out=INVU, in_=U)
        nc.vector.tensor_scalar_mul(INVU, INVU, 0.5)
        nc.vector.tensor_mul(FUU, FPS, U)
        VG = sing.tile([1, 2], F32)
        nc.sync.dma_start(out=VG, in_=cc2_out[:].rearrange("(o c) -> o c", o=1))
        GSC = sing.tile([1, 1], F32)
        nc.vector.tensor_scalar_mul(GSC, VG[0:1, 0:1], invN)
        nc.scalar.activation(out=GSC, in_=GSC, func=AF.Sqrt, bias=BEPSV[:])
        nc.vector.reciprocal(GSC, GSC)
        nc.vector.tensor_mul(GSC, GSC, GAM)
        G28 = sing.tile([NPT, 1], F32)
        nc.gpsimd.partition_broadcast(G28, GSC)
        # warm the PE as soon as AR2 lands (WG2 depends on G28)
        with nc.allow_low_precision("warmup"):
            WG2 = ysqp.tile([128, GROUP_PX], F16, tag="ysq")
            nc.vector.tensor_scalar(out=WG2[0:NPT], in0=FPS[:, 0:GROUP_PX],
                                    scalar1=G28[:, 0:1], scalar2=None, op0=OP.mult)
            for _ in range(14):
                wps = psy.tile([128, GROUP_PX], F32, tag="psy")
                nc.tensor.matmul(wps[:], lhsT=IDENT[0:NPT, :], rhs=WG2[0:NPT],
                                 start=True, stop=True)

        # ================= PHASE 3 =================
        # vn = g*(f*u); w2 = sinh(vn)/u = (exp(vn)-exp(-vn)) * (0.5/u)
        VN = TPS
        nc.vector.tensor_scalar(out=VN, in0=FUU, scalar1=G28[:, 0:1],
                                scalar2=None, op0=OP.mult)
        EXT = STSQ
        nc.scalar.activation(out=EXT, in_=VN, func=AF.Exp)
        nc.scalar.activation(out=PSC, in_=VN, func=AF.Exp, scale=-1.0)
        nc.vector.tensor_sub(EXT, EXT, PSC)                         # 2*sinh
        W2 = MUDOT
        nc.vector.tensor_mul(W2, EXT, INVU)
        with nc.allow_low_precision("fp16 w2 by design"):
            nc.scalar.activation(out=W2B, in_=W2, func=AF.Copy)
        if _CACHE.get("debug"):
            nc.sync.dma_start(out=dbg_ps2[5], in_=W2[:])

        out_flat = [out_d[b_, 1:COUT].rearrange("c h w -> c (h w)")
                    for b_ in range(B_LOC)]

        RSQ2 = MA    # mu-dot accumulator tile is dead after stsq; reuse

        def stage_band(band):
            psl = slice(2 * band, 2 * band + 2)
            HT = stgp.tile([2, GROUPS_PER_BAND, GROUP_PX], F16, tag="htb")
            nc.sync.dma_start(out=HT[0:1, :, :], in_=HPSB[psl, :])
            nc.sync.dma_start(out=HT[1:2, :, :], in_=TPSB[psl, :])
            W2S = stgp.tile([1, GROUPS_PER_BAND, GROUP_PX], F16, tag="w2s")
            nc.sync.dma_start(out=W2S[0:1, :, :], in_=W2B[psl, :])
            W2R = w2rp.tile([128, GROUPS_PER_BAND, GROUP_PX], F16, tag="w2r")
            nc.gpsimd.partition_broadcast(W2R, W2S[0:1, :, :])
            return HT, W2R

        with nc.allow_low_precision("fp16 phase3 by design"):
            # psr (rsq) matmuls trail one 2-group block behind the pr/stt
            # stream so the in-order PE never stalls on sqo
            pend_b = [None]     # (band, blk, sqo, OUTCH)
            stge_map = {}

            def flush_blk():
                bandp, blkp, sqo_t, OUTCH_p = pend_b[0]
                pend_b[0] = None
                bp, rbp = divmod(bandp, BANDS_PER_IMG)
                if blkp == 0:
                    stge_map[bandp] = stgp.tile([3, GROUPS_PER_BAND, GROUP_PX],
                                                F32, tag="stg3", name="STGE")
                STGE = stge_map[bandp]
                for j in range(2):
                    k = blkp * 2 + j
                    psr = psb.tile([1, GROUP_PX], F32, tag="sb")
                    nc.tensor.matmul(psr[:], lhsT=O256Z,
                                     rhs=sqo_t[:, bass.ts(j, GROUP_PX)],
                                     start=True, stop=True)
                    if k % 2 == 0:
                        nc.scalar.activation(out=STGE[0:1, k, :], in_=psr[:],
                                             func=AF.Copy)
                    else:
                        nc.vector.tensor_copy(out=STGE[0:1, k, :], in_=psr[:])
                if blkp == 1:
                    del stge_map[bandp]
                    nc.sync.dma_start(out=RSQ2[2 * bandp:2 * bandp + 2, :],
                                      in_=STGE[0:1, :, :])
                    chp = rbp % OUT_BANDS
                    if chp == OUT_BANDS - 1 or rbp == BANDS_PER_IMG - 1:
                        ncols = (chp + 1) * BAND_ROWS * W
                        col0 = (rbp - chp) * BAND_ROWS * W
                        dst = out_flat[bp][:, col0:col0 + ncols]
                        for i in range(8):
                            p0 = i * 16
                            p1 = min(M, p0 + 16)
                            eng = (nc.gpsimd if i < 4 else
                                   (nc.sync if i < 6 else nc.scalar))
                            eng.dma_start(out=dst[p0:p1, :],
                                          in_=OUTCH_p[p0:p1, 0:ncols])

            nxt = stage_band(0)
            OUTCH = None
            for band in range(NBANDS):
                b, rb = divmod(band, BANDS_PER_IMG)
                HT, W2R = nxt
                if band + 1 < NBANDS:
                    nxt = stage_band(band + 1)
                ch = rb % OUT_BANDS          # position within output chunk
                if ch == 0:
                    OUTCH = outp.tile([128, OUT_COLS], F32, tag="outch")
                for blk in range(2):
                    pr2 = pr2p.tile([128, 2, 512], F32, tag="pr2")
                    for j in range(2):
                        k = blk * 2 + j
                        g = band * GROUPS_PER_BAND + k
                        cols = bass.ts(g, GROUP_PX)
                        nc.tensor.matmul(pr2[:, j, 0:GROUP_PX], lhsT=LR1B,
                                         rhs=HT[:, k, :], start=True, stop=False)
                        nc.tensor.matmul(pr2[:, j, 0:GROUP_PX], lhsT=IDENT,
                                         rhs=YCMB[:, cols], start=False, stop=True)
                    bsl = slice((ch * 4 + blk * 2) * GROUP_PX,
                                (ch * 4 + blk * 2 + 2) * GROUP_PX)
                    nc.vector.scalar_tensor_tensor(
                        out=OUTCH[:, bsl].rearrange("p (b c) -> p b c", b=2),
                        in0=pr2[:, :, 0:GROUP_PX], scalar=0.0,
                        in1=W2R[:, blk * 2:blk * 2 + 2, :],
                        op0=OP.max, op1=OP.mult)
                    sqo = ysqp.tile([128, 2 * GROUP_PX], F16, tag="ysq")
                    nc.scalar.activation(out=sqo, in_=OUTCH[:, bsl],
                                         func=AF.Square, scale=0.0625)
                    if pend_b[0] is not None:
                        flush_blk()
                    pend_b[0] = (band, blk, sqo, OUTCH)
            if pend_b[0] is not None:
                flush_blk()

        # rt = sqrt(1 + sum rs^2) -> channel 0 plane
        RT = T2M1
        nc.scalar.activation(out=RT, in_=RSQ2, func=AF.Sqrt, bias=1.0)
        nc.gpsimd.dma_start(out=out_d[:, 0, :, :], in_=RT)

    nc.compile()
    return nc


def _prep_consts(W):
    W = np.asarray(W, np.float32)
    f16 = np.float16
    w9 = np.zeros((128, 9, 128), np.float32)
    for wi in range(9):
        w9[0:S, wi, 0:M] = W[:, 1 + wi * S:1 + (wi + 1) * S].T
        w9[64:64 + S, wi, 127] = 1.0
    sw3 = np.zeros((128, 8), np.float32)
    sw3[0:M, 0] = W[:, 0]
    sw3[127, 1] = 1.0
    sw3[0:M, 5] = 256.0
    sw3[0:M, 6] = 1.0
    sw3[0:M, 7] = 256.0
    redw = np.zeros((M, 3), np.float32)
    redw[:, 0] = W[:, 0]
    redw[:, 2] = 1.0
    lr1 = np.zeros((2, 128), np.float32)
    lr1[1, 0:M] = W[:, 0]
    ident = np.eye(128, dtype=np.float32)
    w0row = np.zeros((1, 128), np.float32)
    w0row[0, 0:M] = W[:, 0]
    c_w0sq = float(np.float32((W[:, 0].astype(np.float64) ** 2).sum()))
    return (w9.reshape(128, 9 * 128).astype(f16), sw3.astype(f16), redw, w0row,
            lr1.astype(f16), ident.astype(f16), c_w0sq)


def _in_maps(x, W, gamma):
    x = np.ascontiguousarray(np.asarray(x, np.float32))
    gamma = np.asarray(gamma, np.float32)
    w9, sw3, redw, w0row, lr1, ident, c_w0sq = _prep_consts(W)
    if "nc" not in _CACHE:
        _CACHE["c_w0sq"] = c_w0sq
        _CACHE["nc"] = _build_nc()
    maps = []
    for c in range(NCORES):
        maps.append({
            "x": x[c * B_LOC:(c + 1) * B_LOC],
            "w9": w9, "sw3": sw3, "redw": redw, "w0row": w0row,
            "lr1i": lr1, "identi": ident,
            "gamma": gamma,
        })
    return _CACHE["nc"], maps


def kernel(x, W, gamma, beta):
    beta = np.asarray(beta, np.float32)
    gamma = np.asarray(gamma, np.float32)
    assert abs(float(beta[0]) - 1.0) < 1e-6 and np.all(np.abs(beta[1:]) < 1e-6), \
        "kernel specialized for beta == Lorentz origin"
    assert float(gamma[0]) > 0.0
    nc, in_maps = _in_maps(x, W, gamma)
    res = run_bass_kernel_spmd(nc, in_maps, list(range(NCORES)))
    out = np.concatenate([res.results[c]["out"] for c in range(NCORES)], axis=0)
    return out


def run_traced(inputs, tmpdir=None):
    """Run with NTFF tracing; returns (exec_time_ns, BassKernelResults)."""
    nc, in_maps = _in_maps(inputs["x"], inputs["W"], inputs["gamma"])
    res = run_bass_kernel_spmd(nc, in_maps, list(range(NCORES)),
                               trace=True, tmpdir=tmpdir)
    return res.exec_time_ns, res


def simulate(inputs, debug=True):
    """Run the kernel through MultiCoreSim; returns list of per-core output dicts."""
    from concourse.bass_interp import MultiCoreSim
    _CACHE.clear()
    _CACHE["debug"] = debug
    x = np.asarray(inputs["x"], np.float32)
    w9, sw3, redw, w0row, lr1, ident, c_w0sq = _prep_consts(inputs["W"])
    _CACHE["c_w0sq"] = c_w0sq
    nc = _build_nc()
    sim = MultiCoreSim(nc, num_cores=NCORES)
    for c in range(NCORES):
        cs = sim.cores[c]
        cs.tensor("x")[:] = x[c * B_LOC:(c + 1) * B_LOC]
        cs.tensor("w9")[:] = w9
        cs.tensor("sw3")[:] = sw3
        cs.tensor("redw")[:] = redw
        cs.tensor("w0row")[:] = w0row
        cs.tensor("lr1i")[:] = lr1
        cs.tensor("identi")[:] = ident
        cs.tensor("gamma")[:] = np.asarray(inputs["gamma"], np.float32)
    sim.simulate(check_with_hw=False)
    names = ["out"]
    if debug:
        names += ["dbg_ycm", "dbg_ps1", "dbg_ps2", "dbg_mu"]
    return [{n: np.array(sim.cores[c].tensor(n)) for n in names}
            for c in range(NCORES)]


if __name__ == "__main__":
    rng = np.random.default_rng(0)
    x = rng.standard_normal((B_GLOB, CIN, H, W), dtype=np.float32)
    W_ = (rng.standard_normal((M, D), dtype=np.float32) / np.sqrt(D)).astype(np.float32)
    gamma = np.ones((1,), np.float32)
    beta = np.zeros((COUT,), np.float32); beta[0] = 1.0
    out = kernel(x=x, W=W_, gamma=gamma, beta=beta)
    print("out", out.shape, out.dtype, np.abs(out).max())
